# revision 1
# baseline (speedup 1.0000x reference)
"""GAT message-passing kernel for Trainium2 (8 NeuronCores, data-parallel over batch).

Math (per batch element b, derived from the reference nn.Module):
    x      = nodes.reshape(N, D)
    self_e = mlp2(x, self_*)                 # [N, H]
    nb_e   = mlp2(x, nb_*)                   # [N, H]
    U      = self_e @ comb_w1[:H]            # [N, H]  (i side)
    V      = nb_e @ comb_w1[H:] + comb_b1    # [N, H]  (j side)
    scores(i,j) = leaky(U_i + V_j) @ w2 + b2
                = 0.8*relu(U_i+V_j)@w2 + 0.2*(sU_i + sV_j) + const_i
    Softmax over j is invariant to per-i constants, so only
      s'(i,j) = 0.8*relu(U_i+V_j)@w2 + 0.2*sV_j  matters, and
      exp(s') factorizes as exp(0.8 relu(...)@w2) * exp(0.2 sV_j).
    E^T[j,i] = edges[j,i]*(j!=i)*exp(0.2 sV_j) * exp(0.8 relu(U_i+V_j)@w2)
    denom[i] = sum_j E^T[j,i]; gate = denom > eps; recip = gate/denom
    out[i]   = gate * (recip * (E^T)^T @ nb_e + self_e)
    (|scores| < 2, so exp needs no max-subtraction.)

Device mapping (one core per batch element; ~79us/core in the CoreSim model,
PE-bound):
  - Transposed (g,h)-on-partitions layout: partitions = (i-parity g, h), so one
    DVE/ACT/gpsimd tensor_scalar(add,max) op builds relu(V + U_i) for TWO i's
    at once as a [128, 512] bf16 tile; the 256 builds round-robin over the
    three engines (BUILD_PATTERN).
  - PE reduces over (g,h) with a slotted block-diagonal 0.8*w2 lhsT: M=64,
    2 column groups at PSUM base partitions {0, 64}, 32 accumulating matmuls
    per group -> scores for a 128-i tile land as one [128, 512] PSUM bank.
    The 32 slot matrices are windows bigW[:, 62-2s : 126-2s] of a single
    [128, 128] buffer with the two nonzero columns at 62:64.
  - ACT applies exp straight out of PSUM (bf16 out); PE transposes 128x128
    chunks; DVE multiplies by mask tiles (edges * (1-eye) * exp(0.2 sV_j),
    built in the natural [j, i] layout - no mask transpose) -> E^T.
  - Per i-tile, PE also runs the ones-matmul denom partials and the
    E^T @ nb_e aggregation; the [1,128] denom row becomes [128,1] per-partition
    scalars via a tiny SBUF->DRAM->SBUF scatter, so recip/gate/output assembly
    pipeline inside the main loop.
  - MLP/U/V precompute stays fp32 (self_e adds directly into the output, so
    bf16 there blows up small-element relative error); the pairwise stage is
    bf16 (absmax-rel err 1.7e-4, resid_var 4e-8 vs fp64 reference).
"""

import os
import sys

sys.path.insert(0, "/opt/trn_rl_repo")

import numpy as np
import ml_dtypes

import concourse.bass as bass
import concourse.bacc as bacc
import concourse.tile as tile
from concourse import mybir, bass2jax
from concourse.bass_utils import run_bass_kernel_spmd

B, N, H, D = 8, 512, 64, 128
NCORES = 8
NT = N // 128          # 4 i/j tiles of 128
NPAIR = N // 2         # 256 i-pairs
SLOTS = 32             # i-pairs per 64-partition column group
F32 = mybir.dt.float32
BF16 = mybir.dt.bfloat16
I32 = mybir.dt.int32

# Build-engine schedule for the 256 relu'd tiles: 'v' = VectorE, 'a' = ScalarE,
# 'p' = gpsimd/Pool. Tuned from profiles.
BUILD_PATTERN = os.environ.get("GAT_BUILD_PATTERN", "vpvavpvvpvavpvvpavvpvavpvvpvavpv")

_CACHE = {}


def _build_module():
    nc = bacc.Bacc("TRN2", target_bir_lowering=False, debug=False, num_devices=NCORES)

    # ---- per-core data ----
    nodes = nc.dram_tensor("nodes", [N, D], F32, kind="ExternalInput")
    edges = nc.dram_tensor("edges", [N, N], mybir.dt.uint8, kind="ExternalInput")
    # ---- parameters / host-prepared constants (same on all cores) ----
    w1_self = nc.dram_tensor("w1_self", [D, H], F32, kind="ExternalInput")
    w2_self = nc.dram_tensor("w2_self", [H, H], F32, kind="ExternalInput")
    w1_nb = nc.dram_tensor("w1_nb", [D, H], F32, kind="ExternalInput")
    w2_nb = nc.dram_tensor("w2_nb", [H, H], F32, kind="ExternalInput")
    w1_cs = nc.dram_tensor("w1_cs", [H, H], F32, kind="ExternalInput")
    w1_cn = nc.dram_tensor("w1_cn", [H, H], F32, kind="ExternalInput")
    w2_c = nc.dram_tensor("w2_c", [H, 1], BF16, kind="ExternalInput")
    bvec = nc.dram_tensor("bvec", [H, 5], F32, kind="ExternalInput")
    rowpack = nc.dram_tensor("rowpack", [1, 256], F32, kind="ExternalInput")
    id_f32 = nc.dram_tensor("id_f32", [128, 128], F32, kind="ExternalInput")
    id_bf16 = nc.dram_tensor("id_bf16", [128, 128], BF16, kind="ExternalInput")
    w2bdpack = nc.dram_tensor("w2bdpack", [128, 2], BF16, kind="ExternalInput")
    inveye = nc.dram_tensor("inveye", [128, 128], BF16, kind="ExternalInput")

    out = nc.dram_tensor("out", [N, H], F32, kind="ExternalOutput")

    scr_den = nc.dram_tensor("scr_den", [N], F32)
    scr_sv = nc.dram_tensor("scr_sv", [N], F32)

    with tile.TileContext(nc) as tc:
        _emit(nc, tc, locals())
    nc.compile()
    return nc


def _emit(nc, tc, t):
    AF = mybir.ActivationFunctionType
    OP = mybir.AluOpType

    with (
        tc.tile_pool(name="persist", bufs=1) as P,
        tc.tile_pool(name="xwork", bufs=2) as XW,
        tc.tile_pool(name="ework", bufs=2) as EW,
        tc.tile_pool(name="relu", bufs=18) as RL,
        tc.tile_pool(name="xexp", bufs=4) as XE,
        tc.tile_pool(name="small", bufs=4) as SM,
        tc.tile_pool(name="psumR", bufs=3, space="PSUM") as PR,
        tc.tile_pool(name="psumT", bufs=3, space="PSUM") as PT,
        tc.tile_pool(name="psumM", bufs=1, space="PSUM") as PM,
        tc.tile_pool(name="psumA", bufs=1, space="PSUM") as PA,
    ):
        # ---------- load constants ----------
        def load(name, shape, dtype, eng=None):
            tl = P.tile(shape, dtype, tag=name)
            (eng or nc.sync).dma_start(out=tl[:], in_=t[name].ap())
            return tl

        xins = []
        for it in range(NT):
            xin = XW.tile([128, D], F32, name="xin", tag="xin")
            nc.sync.dma_start(out=xin[:], in_=t["nodes"].ap()[bass.ts(it, 128), :])
            xins.append(xin)
        w1s = load("w1_self", [D, H], F32, eng=nc.scalar)
        w1n = load("w1_nb", [D, H], F32, eng=nc.scalar)
        w2s = load("w2_self", [H, H], F32, eng=nc.scalar)
        w2n = load("w2_nb", [H, H], F32, eng=nc.scalar)
        w1cs = load("w1_cs", [H, H], F32, eng=nc.scalar)
        w1cn = load("w1_cn", [H, H], F32, eng=nc.scalar)
        w2cb = load("w2_c", [H, 1], BF16, eng=nc.scalar)
        bvec = load("bvec", [H, 5], F32, eng=nc.sync)
        b1s, b1n = bvec[:, 0:1], bvec[:, 1:2]
        b2sc, b2nc, b1c = bvec[:, 2:3], bvec[:, 3:4], bvec[:, 4:5]
        rowp = load("rowpack", [1, 256], F32, eng=nc.sync)
        onesr, b2sr, b2nr = rowp[:, 0:128], rowp[:, 128:192], rowp[:, 192:256]
        idf = P.tile([128, 128], F32, tag="id_f32")
        nc.gpsimd.dma_start(out=idf[:], in_=t["id_f32"].ap())
        idb = P.tile([128, 128], BF16, tag="id_bf16")
        nc.gpsimd.dma_start(out=idb[:], in_=t["id_bf16"].ap())
        onesc = P.tile([128, 1], BF16, tag="onesc")
        nc.gpsimd.memset(onesc[:], 1.0)
        ive = P.tile([128, 128], BF16, tag="ive")
        nc.gpsimd.dma_start(out=ive[:], in_=t["inveye"].ap())
        w2bd_all = P.tile([128, 128], BF16, tag="w2bd_all")
        nc.gpsimd.memset(w2bd_all[:], 0.0)
        nc.gpsimd.dma_start(out=w2bd_all[:, 62:64], in_=t["w2bdpack"].ap())
        w2bd_sb = [w2bd_all[:, 62 - 2 * s:126 - 2 * s] for s in range(SLOTS)]

        # ---------- x -> x^T (bf16 for the small MLP matmuls) ----------
        xT = P.tile([D, N], F32, tag="xT")
        for it in range(NT):
            px = PT.tile([128, 128], F32, tag="pt", name="px", padded_shape=[128, 128])
            nc.tensor.transpose(px[:], xins[it][:], idf[:])
            nc.vector.tensor_copy(out=xT[:, bass.ts(it, 128)], in_=px[:])

        # ---------- tiny MLPs (transposed; h on partitions) ----------
        def leaky_from_psum(psum, bias, tag):
            z = EW.tile([H, N], F32, tag="lk_z")
            nc.scalar.activation(out=z[:], in_=psum[:H, :], func=AF.Identity,
                                 bias=bias, scale=1.0)
            h1 = P.tile([H, N], F32, tag=tag, name=tag)
            nc.vector.scalar_tensor_tensor(out=h1[:], in0=z[:], scalar=0.2,
                                           in1=z[:], op0=OP.mult, op1=OP.max)
            return h1

        pm = PM.tile([128, N], F32, tag="mm")
        nc.tensor.matmul(pm[:H, :], w1n[:], xT[:], start=True, stop=True)
        h1T_n = leaky_from_psum(pm, b1n, "h1T_n")

        pm = PM.tile([128, N], F32, tag="mm")
        nc.tensor.matmul(pm[:H, :], w2n[:], h1T_n[:], start=True, stop=True)
        eT_n = P.tile([H, N], F32, tag="eT_n")
        nc.scalar.activation(out=eT_n[:], in_=pm[:H, :], func=AF.Identity,
                             bias=b2nc, scale=1.0)

        # Vrep (bf16, both partition halves) written straight from PSUM on
        # parallel engines; no intermediate f32 V^T tile
        pm = PM.tile([128, N], F32, tag="mm")
        nc.tensor.matmul(pm[:H, :], w1cn[:], eT_n[:], start=True, stop=True)
        Vrep = P.tile([128, N], BF16, tag="Vrep")
        nc.scalar.activation(out=Vrep[:H, :], in_=pm[:H, :], func=AF.Identity,
                             bias=b1c, scale=1.0)
        nc.vector.tensor_scalar_add(out=Vrep[H:, :], in0=pm[:H, :], scalar1=b1c)

        # self chain, chunked by 128 i-columns so U2's early columns land
        # early (first builds only need U2[:, 0:64])
        h1T_s = P.tile([H, N], F32, tag="h1T_s")
        eT_s = P.tile([H, N], F32, tag="eT_s")
        U2 = P.tile([128, NPAIR], F32, tag="U2")
        for it in range(NT):
            cs = bass.ts(it, 128)
            pm = PT.tile([128, 128], F32, tag="pt", name="pmc")
            nc.tensor.matmul(pm[:H, :], w1s[:], xT[:, cs], start=True, stop=True)
            zc = EW.tile([H, 128], F32, tag="lk_zc", name="zc")
            nc.scalar.activation(out=zc[:], in_=pm[:H, :], func=AF.Identity,
                                 bias=b1s, scale=1.0)
            nc.vector.scalar_tensor_tensor(out=h1T_s[:, cs], in0=zc[:], scalar=0.2,
                                           in1=zc[:], op0=OP.mult, op1=OP.max)
            pm = PT.tile([128, 128], F32, tag="pt", name="pmc")
            nc.tensor.matmul(pm[:H, :], w2s[:], h1T_s[:, cs], start=True, stop=True)
            nc.scalar.activation(out=eT_s[:, cs], in_=pm[:H, :], func=AF.Identity,
                                 bias=b2sc, scale=1.0)
            pm = PT.tile([128, 128], F32, tag="pt", name="pmc")
            nc.tensor.matmul(pm[:H, :], w1cs[:], eT_s[:, cs], start=True, stop=True)
            psplit = pm[:H, :].rearrange("p (i g) -> p i g", g=2)
            nc.vector.tensor_copy(out=U2[:H, bass.ts(it, 64)], in_=psplit[:, :, 0])
            nc.vector.tensor_copy(out=U2[H:, bass.ts(it, 64)], in_=psplit[:, :, 1])

        # exp(0.2 * sV) row -> scatter to [128, NT] per-partition scalars
        pm = PM.tile([128, N], F32, tag="mm")
        nc.tensor.matmul(pm[:1, :], w2cb[:], Vrep[:H, :], start=True, stop=True)
        sv_row = SM.tile([1, N], F32, tag="sv_row")
        nc.scalar.activation(out=sv_row[:], in_=pm[:1, :], func=AF.Exp, scale=0.2)
        pesv = PT.tile([128, 128], F32, tag="pt", name="pesv", padded_shape=[128, 128])
        for tq in range(NT):
            nc.tensor.transpose(pesv[:, tq:tq + 1], sv_row[:, bass.ts(tq, 128)],
                                idf[0:1, 0:1])
        esv = P.tile([128, NT], F32, tag="esv")
        nc.vector.tensor_copy(out=esv[:], in_=pesv[:, 0:NT])

        # ---------- mask tiles: edges * notdiag * exp(0.2 sV_j) ----------
        masks = []
        for jt in range(NT):
            esb = EW.tile([128, N], mybir.dt.uint8, tag="edges_in")
            nc.gpsimd.dma_start(out=esb[:], in_=t["edges"].ap()[bass.ts(jt, 128), :])
            mj = P.tile([128, N], BF16, tag=f"mask{jt}", name=f"mask{jt}")
            nc.vector.tensor_scalar_mul(out=mj[:], in0=esb[:], scalar1=esv[:, jt:jt + 1])
            nc.vector.tensor_mul(out=mj[:, bass.ts(jt, 128)], in0=mj[:, bass.ts(jt, 128)],
                                 in1=ive[:])
            masks.append(mj)

        # ---------- self_e / nb_e in [row, H] layout ----------
        selfe, nbe = [], []
        for it in range(NT):
            pa = PT.tile([128, H], F32, tag="pt", name="pa", padded_shape=[128, 128])
            nc.tensor.matmul(pa[:], h1T_s[:, bass.ts(it, 128)], w2s[:], start=True, stop=False)
            nc.tensor.matmul(pa[:], onesr, b2sr, start=False, stop=True)
            se = P.tile([128, H], F32, tag=f"selfe{it}")
            nc.scalar.copy(out=se[:], in_=pa[:])
            selfe.append(se)
        for jt in range(NT):
            pa = PT.tile([128, H], F32, tag="pt", name="pa", padded_shape=[128, 128])
            nc.tensor.matmul(pa[:], h1T_n[:, bass.ts(jt, 128)], w2n[:], start=True, stop=False)
            nc.tensor.matmul(pa[:], onesr, b2nr, start=False, stop=True)
            ne = P.tile([128, H], BF16, tag=f"nbe{jt}")
            nc.scalar.copy(out=ne[:], in_=pa[:])
            nbe.append(ne)

        # ---------- main pass: scores -> exp -> E^T -> denom/agg ----------
        ET = [P.tile([128, N], BF16, tag=f"ET{jt}", name=f"ET{jt}") for jt in range(NT)]
        pat = BUILD_PATTERN
        pd = PM.tile([128, N], F32, tag="mm")
        pa_all = PA.tile([128, NT, H], F32, tag="pa_all")
        for it in range(NT):
            ps = PR.tile([128, N], F32, tag="psumR")
            for c in range(2):
                for s in range(SLOTS):
                    p = 64 * it + 32 * c + s
                    rl = RL.tile([128, N], BF16, tag="relu")
                    eng = pat[p % len(pat)]
                    if eng == "v":
                        nc.vector.tensor_scalar(out=rl[:], in0=Vrep[:],
                                                scalar1=U2[:, p:p + 1], scalar2=0.0,
                                                op0=OP.add, op1=OP.max)
                    elif eng == "a":
                        nc.scalar.activation(out=rl[:], in_=Vrep[:], func=AF.Relu,
                                             bias=U2[:, p:p + 1], scale=1.0)
                    else:
                        nc.gpsimd.tensor_scalar(out=rl[:], in0=Vrep[:],
                                                scalar1=U2[:, p:p + 1], scalar2=0.0,
                                                op0=OP.add, op1=OP.max)
                    nc.tensor.matmul(ps[bass.ts(c, 64), :], w2bd_sb[s], rl[:],
                                     start=(s == 0), stop=(s == SLOTS - 1))
            X = XE.tile([128, N], BF16, tag="X")
            nc.scalar.activation(out=X[:], in_=ps[:], func=AF.Exp)
            for jt in range(NT):
                px = PT.tile([128, 128], BF16, tag="pt")
                nc.tensor.transpose(px[:], X[:, bass.ts(jt, 128)], idb[:])
                nc.vector.tensor_mul(out=ET[jt][:, bass.ts(it, 128)], in0=px[:],
                                     in1=masks[jt][:, bass.ts(it, 128)])
            # denom partial: accumulate sum_j over this it's column block
            for jt in range(NT):
                nc.tensor.matmul(pd[:1, bass.ts(it, 128)], onesc[:],
                                 ET[jt][:, bass.ts(it, 128)],
                                 start=(jt == 0), stop=(jt == NT - 1))
            # aggregation for this i-tile
            for jt in range(NT):
                nc.tensor.matmul(pa_all[:, it, :], ET[jt][:, bass.ts(it, 128)], nbe[jt][:],
                                 start=(jt == 0), stop=(jt == NT - 1))
            # denom [1,128] row -> [128,1] column via a K=1 PE transpose
            # (stays on-chip; replaces a 2-DMA DRAM roundtrip)
            den_row = SM.tile([1, 128], F32, tag="den_row")
            nc.vector.tensor_copy(out=den_row[:], in_=pd[:1, bass.ts(it, 128)])
            pden = PT.tile([128, 128], F32, tag="pt", name="pden", padded_shape=[128, 128])
            nc.tensor.transpose(pden[:, 0:1], den_row[:], idf[0:1, 0:1])
            gate = SM.tile([128, 1], F32, tag="gate", name="gate")
            nc.vector.tensor_single_scalar(out=gate[:], in_=pden[:, 0:1], scalar=1e-6, op=OP.is_gt)
            dsafe = SM.tile([128, 1], F32, tag="dsafe", name="dsafe")
            nc.vector.tensor_scalar_max(out=dsafe[:], in0=pden[:, 0:1], scalar1=1e-30)
            recipg = SM.tile([128, 1], F32, tag="recipg", name="recipg")
            nc.vector.reciprocal(out=recipg[:], in_=dsafe[:])
            # output assembly for this i-tile
            sg = SM.tile([128, H], F32, tag="sg")
            nc.vector.tensor_scalar_mul(out=sg[:], in0=selfe[it][:], scalar1=gate[:])
            nc.vector.tensor_mul(out=recipg[:], in0=recipg[:], in1=gate[:])
            ot = SM.tile([128, H], F32, tag="ot")
            nc.vector.scalar_tensor_tensor(out=ot[:], in0=pa_all[:, it, :],
                                           scalar=recipg[:], in1=sg[:],
                                           op0=OP.mult, op1=OP.add)
            nc.sync.dma_start(out=t["out"].ap()[bass.ts(it, 128), :], in_=ot[:])


def _host_constants(inputs):
    f32 = np.float32
    bf = ml_dtypes.bfloat16
    H_ = H
    w2 = np.asarray(inputs["comb_w2"], f32)            # [H, 1]
    w2bdpack = np.zeros((128, 2), f32)
    w2bdpack[0:H_, 0] = 0.8 * w2[:, 0]
    w2bdpack[H_:128, 1] = 0.8 * w2[:, 0]
    ive = (1.0 - np.eye(128)).astype(f32)
    consts = {
        "w1_self": np.asarray(inputs["self_w1"], f32),
        "w2_self": np.asarray(inputs["self_w2"], f32),
        "w1_nb": np.asarray(inputs["nb_w1"], f32),
        "w2_nb": np.asarray(inputs["nb_w2"], f32),
        "w1_cs": np.ascontiguousarray(np.asarray(inputs["comb_w1"], f32)[:H_]),
        "w1_cn": np.ascontiguousarray(np.asarray(inputs["comb_w1"], f32)[H_:]),
        "w2_c": w2.astype(bf),
        "bvec": np.stack([
            np.asarray(inputs["self_b1"], f32),
            np.asarray(inputs["nb_b1"], f32),
            np.asarray(inputs["self_b2"], f32),
            np.asarray(inputs["nb_b2"], f32),
            np.asarray(inputs["comb_b1"], f32),
        ], axis=1),
        "rowpack": np.concatenate([
            np.ones(128, f32),
            np.asarray(inputs["self_b2"], f32),
            np.asarray(inputs["nb_b2"], f32),
        ]).reshape(1, 256),
        "id_f32": np.eye(128, dtype=f32),
        "id_bf16": np.eye(128, dtype=f32).astype(bf),
        "w2bdpack": w2bdpack.astype(bf),
        "inveye": ive.astype(bf),
    }
    return consts


def _build_fast_path(nc):
    """Cache a single jitted shard_map executable so repeat kernel() calls
    skip jax re-tracing (same lowering run_bass_kernel_spmd uses under axon)."""
    import jax
    from jax.sharding import Mesh, PartitionSpec
    from jax.experimental.shard_map import shard_map

    bass2jax.install_neuronx_cc_hook()
    pname = nc.partition_id_tensor.name if nc.partition_id_tensor else None
    in_names, out_names, out_avals = [], [], []
    for alloc in nc.m.functions[0].allocations:
        if not isinstance(alloc, mybir.MemoryLocationSet):
            continue
        name = alloc.memorylocations[0].name
        if alloc.kind == "ExternalInput":
            if name != pname:
                in_names.append(name)
        elif alloc.kind == "ExternalOutput":
            out_names.append(name)
            out_avals.append(jax.core.ShapedArray(tuple(alloc.tensor_shape),
                                                  mybir.dt.np(alloc.dtype)))
    all_names = in_names + out_names + ([pname] if pname else [])

    def _body(*args):
        operands = list(args)
        if pname is not None:
            operands.append(bass2jax.partition_id_tensor())
        return tuple(bass2jax._bass_exec_p.bind(
            *operands, out_avals=tuple(out_avals), in_names=tuple(all_names),
            out_names=tuple(out_names), lowering_input_output_aliases=(),
            sim_require_finite=True, sim_require_nnan=True, nc=nc))

    devices = jax.devices()[:NCORES]
    mesh = Mesh(np.asarray(devices), ("core",))
    n_io = len(in_names) + len(out_names)
    sharded = jax.jit(
        shard_map(_body, mesh=mesh, in_specs=(PartitionSpec("core"),) * n_io,
                  out_specs=(PartitionSpec("core"),) * len(out_names),
                  check_rep=False),
        keep_unused=True,
    )
    return sharded, in_names, out_names, out_avals


def kernel(**inputs):
    first = "nc" not in _CACHE
    if first:
        _CACHE["nc"] = _build_module()
    nc = _CACHE["nc"]

    consts = _host_constants(inputs)
    nodes = np.asarray(inputs["nodes"], np.float32).reshape(B, N, D)
    edges = (np.asarray(inputs["edges"]) != 0).astype(np.uint8)

    in_maps = []
    for c in range(NCORES):
        m = dict(consts)
        m["nodes"] = np.ascontiguousarray(nodes[c])
        m["edges"] = edges[c]
        in_maps.append(m)

    if first:
        res = run_bass_kernel_spmd(nc, in_maps, core_ids=list(range(NCORES)))
        _CACHE["fast"] = _build_fast_path(nc)
        return np.stack([res.results[c]["out"] for c in range(NCORES)]).astype(np.float32)

    import jax
    sharded, in_names, out_names, out_avals = _CACHE["fast"]
    ckey = hash(tuple((k, v.tobytes()) for k, v in sorted(consts.items())))
    if _CACHE.get("ckey") != ckey:
        _CACHE["cdev"] = {
            n: jax.device_put(np.concatenate([np.asarray(in_maps[c][n])
                                              for c in range(NCORES)], axis=0))
            for n in in_names if n not in ("nodes", "edges")
        }
        _CACHE["zdev"] = [jax.device_put(np.zeros((NCORES * a.shape[0], *a.shape[1:]),
                                                  a.dtype)) for a in out_avals]
        _CACHE["ckey"] = ckey
    cdev = _CACHE["cdev"]
    concat_in = [cdev[n] if n in cdev else
                 np.concatenate([np.asarray(in_maps[c][n]) for c in range(NCORES)], axis=0)
                 for n in in_names]
    outs = sharded(*concat_in, *_CACHE["zdev"])
    i = out_names.index("out")
    return np.asarray(outs[i]).reshape(NCORES, N, H).astype(np.float32)



# revision 7
# speedup vs baseline: 1.2502x; 1.2502x over previous
"""GAT message-passing kernel for Trainium2 (8 NeuronCores, data-parallel over batch).

Math (per batch element b, derived from the reference nn.Module):
    x      = nodes.reshape(N, D)
    self_e = mlp2(x, self_*)                 # [N, H]
    nb_e   = mlp2(x, nb_*)                   # [N, H]
    U      = self_e @ comb_w1[:H]            # [N, H]  (i side)
    V      = nb_e @ comb_w1[H:] + comb_b1    # [N, H]  (j side)
    scores(i,j) = leaky(U_i + V_j) @ w2 + b2
                = 0.8*relu(U_i+V_j)@w2 + 0.2*(sU_i + sV_j) + const_i
    Softmax over j is invariant to per-i constants, so only
      s'(i,j) = 0.8*relu(U_i+V_j)@w2 + 0.2*sV_j  matters, and
      exp(s') factorizes as exp(0.8 relu(...)@w2) * exp(0.2 sV_j).
    E^T[j,i] = edges[j,i]*(j!=i)*exp(0.2 sV_j) * exp(0.8 relu(U_i+V_j)@w2)
    denom[i] = sum_j E^T[j,i]; gate = denom > eps; recip = gate/denom
    out[i]   = gate * (recip * (E^T)^T @ nb_e + self_e)
    (|scores| < 2, so exp needs no max-subtraction.)

Device mapping (one core per batch element). The pairwise stage uses the
transposed (g,h)-on-partitions layout: partitions = (i-parity g, h), free = j,
so one tensor_scalar(add,max)/activation(Relu,bias) op builds relu(V + U_i)
for TWO i's at once as a [128, 512] tile. Per 16 slot-pairs (one 64-row PSUM
column group), a pattern string assigns each slot-pair one of:
  'v'  two bf16 builds on DVE (4x perf mode, ~194ns) + two bf16 slot matmuls
       (512 rows * 1 cyc = ~213ns each) using shifted block-diagonal 0.8*w2
       windows;
  'a'/'p'/'V' two fp8e4m3 builds on ACT/Pool/DVE + ONE DoubleRow fp8 matmul
       covering both i-pairs in 256 cycles (~107ns) — 4x PE throughput per
       pair vs bf16;
  'h'/'w'/'x' mixed-engine fp8 builds (ACT+Pool / ACT+DVE / Pool+DVE) + DR.
fp8 relu tiles + fp8 0.8*w2 quantization costs ~8e-4 output rel err (checked
against the fp64 reference; budget is 2e-2).

MLP/U/V precompute runs in fp32r (1 cyc/row at >=256 free vs 4 for fp32) for
the self chain (self_e adds into the output, needs f32 accuracy) and bf16 for
the neighbor chain, both chunked by 256 columns so the first U2/Vrep columns
land early. Denominators, aggregation and output assembly are unchanged from
the bf16 scheme: exp straight out of PSUM, PE transposes, DVE mask-muls,
ones-matmul denoms, E^T @ nb_e aggregation, K=1 PE transpose for the
[1,128]->[128,1] denom scatter. The main loop is software-pipelined: the
post-stage (exp/ET/denom/agg/assembly) of i-tile it-1 is emitted between the
two column groups of i-tile it, which keeps the in-order DVE/ACT queues from
stalling on X(it-1).
"""

import os
import sys

sys.path.insert(0, "/opt/trn_rl_repo")

import numpy as np
import ml_dtypes

import concourse.bass as bass
import concourse.bacc as bacc
import concourse.tile as tile
from concourse import mybir, bass2jax
from concourse.bass_utils import run_bass_kernel_spmd

B, N, H, D = 8, 512, 64, 128
NCORES = 8
NT = N // 128          # 4 i/j tiles of 128
NPAIR = N // 2         # 256 i-pairs
F32 = mybir.dt.float32
F32R = mybir.dt.float32r
BF16 = mybir.dt.bfloat16
FP8 = mybir.dt.float8e4
U8 = mybir.dt.uint8

# Per-column-group slot-pair engine pattern (16 chars, order irrelevant):
# 'v' bf16 DVE; 'a' fp8 ACT; 'p' fp8 Pool; 'V' fp8 DVE; 'h' ACT+Pool;
# 'w' ACT+DVE; 'x' Pool+DVE.
PAIR_PATTERN = os.environ.get("GAT_PAIR_PATTERN", "vvvvvvvvvvaaahpp")

_CACHE = {}


def _build_module():
    nc = bacc.Bacc("TRN2", target_bir_lowering=False, debug=False, num_devices=NCORES)

    nodes = nc.dram_tensor("nodes", [N, D], F32R, kind="ExternalInput")
    edges = nc.dram_tensor("edges", [N, N], U8, kind="ExternalInput")
    wpack = nc.dram_tensor("wpack", [128, 192], F32R, kind="ExternalInput")
    bvec = nc.dram_tensor("bvec", [64, 5], F32, kind="ExternalInput")
    id_f32 = nc.dram_tensor("id_f32", [128, 128], F32R, kind="ExternalInput")
    bfpack = nc.dram_tensor("bfpack", [128, 640], BF16, kind="ExternalInput")
    w2drall = nc.dram_tensor("w2drall", [128, 2048], FP8, kind="ExternalInput")

    out = nc.dram_tensor("out", [N, H], F32, kind="ExternalOutput")

    with tile.TileContext(nc) as tc:
        _emit(nc, tc, locals())
    nc.compile()
    return nc


def _emit(nc, tc, t):
    AF = mybir.ActivationFunctionType
    OP = mybir.AluOpType
    DRMODE = mybir.MatmulPerfMode.DoubleRow
    PAT = PAIR_PATTERN
    assert len(PAT) == 16 and all(c in "vapVhwx" for c in PAT), PAT
    # builds per fp8 slot-pair: (engine_t0, engine_t1)
    FP8_ENG = {"a": "AA", "p": "PP", "V": "DD", "h": "AP", "w": "AD", "x": "PD"}

    with (
        tc.tile_pool(name="persist", bufs=1) as P,
        tc.tile_pool(name="xin", bufs=2) as XW,
        tc.tile_pool(name="ework", bufs=3) as EW,
        tc.tile_pool(name="edges", bufs=4) as EB,
        tc.tile_pool(name="relu", bufs=12) as RL,
        tc.tile_pool(name="relu8", bufs=8) as R8,
        tc.tile_pool(name="xexp", bufs=2) as XE,
        tc.tile_pool(name="small", bufs=4) as SM,
        tc.tile_pool(name="psumR", bufs=2, space="PSUM") as PR,
        tc.tile_pool(name="psumT", bufs=2, space="PSUM") as PT,
        tc.tile_pool(name="psumM", bufs=2, space="PSUM") as PM,
        tc.tile_pool(name="psumD", bufs=1, space="PSUM") as PD,
        tc.tile_pool(name="psumA", bufs=1, space="PSUM") as PA,
    ):
        # ---------- input DMAs (all on the idle SP queue, in need-order) ----
        xins = []
        for it in range(NT):
            xin = XW.tile([128, D], F32R, name="xin", tag="xin")
            nc.sync.dma_start(out=xin[:], in_=t["nodes"].ap()[bass.ts(it, 128), :])
            xins.append(xin)
        wp = P.tile([128, 192], F32R, tag="wpack")
        nc.sync.dma_start(out=wp[:], in_=t["wpack"].ap())
        bv = P.tile([64, 5], F32, tag="bvec")
        nc.sync.dma_start(out=bv[:], in_=t["bvec"].ap())
        idf = P.tile([128, 128], F32R, tag="id_f32")
        nc.sync.dma_start(out=idf[:], in_=t["id_f32"].ap())
        bp = P.tile([128, 640], BF16, tag="bfpack")
        nc.sync.dma_start(out=bp[:], in_=t["bfpack"].ap())
        w2dr = P.tile([128, 2048], FP8, tag="w2drall")
        nc.sync.dma_start(out=w2dr[:], in_=t["w2drall"].ap())
        esbs = []
        for jt in range(NT):
            esb = EB.tile([128, N], U8, tag="edges_in", name=f"esb{jt}")
            nc.sync.dma_start(out=esb[:], in_=t["edges"].ap()[bass.ts(jt, 128), :])
            esbs.append(esb)

        # ---------- constant views ----------
        w1s, w2s, w1cs = wp[:, 0:64], wp[0:64, 64:128], wp[0:64, 128:192]
        b1s, b1n = bv[:, 0:1], bv[:, 1:2]
        b2sc, b2nc, b1c = bv[:, 2:3], bv[:, 3:4], bv[:, 4:5]
        w1n, w2n, w1cn = bp[:, 0:64], bp[0:64, 64:128], bp[0:64, 128:192]
        w2cb = bp[0:64, 192:193]
        idb = bp[:, 193:321]
        ive = bp[:, 321:449]
        w2bd_sb = [bp[:, 449 + 62 - 2 * s: 449 + 126 - 2 * s] for s in range(32)]
        onesc = P.tile([128, 1], BF16, tag="onesc")
        nc.gpsimd.memset(onesc[:], 1.0)

        # ---------- x -> x^T (f32r transposes: 1.5 cyc/row) ----------
        xT = P.tile([D, N], F32R, tag="xT")
        for it in range(NT):
            px = PT.tile([128, 128], F32R, tag="pt", name="px", padded_shape=[128, 128])
            nc.tensor.transpose(px[:], xins[it][:], idf[:])
            eng = nc.vector if it % 2 == 0 else nc.gpsimd
            eng.tensor_copy(out=xT[:, bass.ts(it, 128)], in_=px[:])

        # ---------- tiny MLPs, chunked by 256 cols (h on partitions) --------
        # nb chain in bf16, self chain in f32r (self_e adds into the output).
        h1T_n = P.tile([H, N], BF16, tag="h1T_n")
        h1T_s = P.tile([H, N], F32R, tag="h1T_s")
        eT_n = P.tile([H, N], BF16, tag="eT_n")
        eT_s = P.tile([H, N], F32R, tag="eT_s")
        Vrep = P.tile([128, N], BF16, tag="Vrep")
        U2 = P.tile([128, NPAIR], F32, tag="U2")

        for k in range(2):
            cs = bass.ts(k, 256)
            pm = PM.tile([64, 256], F32, tag="pm", name="pm_w1n")
            nc.tensor.matmul(pm[:], w1n, xT[:, cs], start=True, stop=True)
            zn = EW.tile([H, 256], BF16, tag="zn", name="zn")
            nc.scalar.activation(out=zn[:], in_=pm[:], func=AF.Identity,
                                 bias=b1n, scale=1.0)
            nc.vector.scalar_tensor_tensor(out=h1T_n[:, cs], in0=zn[:], scalar=0.2,
                                           in1=zn[:], op0=OP.mult, op1=OP.max)
            pm = PM.tile([64, 256], F32, tag="pm", name="pm_w1s")
            nc.tensor.matmul(pm[:], w1s, xT[:, cs], start=True, stop=True)
            zs = EW.tile([H, 256], F32, tag="zs", name="zs")
            nc.scalar.activation(out=zs[:], in_=pm[:], func=AF.Identity,
                                 bias=b1s, scale=1.0)
            nc.vector.scalar_tensor_tensor(out=h1T_s[:, cs], in0=zs[:], scalar=0.2,
                                           in1=zs[:], op0=OP.mult, op1=OP.max)

        for k in range(2):
            cs = bass.ts(k, 256)
            pm = PM.tile([64, 256], F32, tag="pm", name="pm_w2n")
            nc.tensor.matmul(pm[:], w2n, h1T_n[:, cs], start=True, stop=True)
            nc.scalar.activation(out=eT_n[:, cs], in_=pm[:], func=AF.Identity,
                                 bias=b2nc, scale=1.0)
            pm = PM.tile([64, 256], F32, tag="pm", name="pm_w2s")
            nc.tensor.matmul(pm[:], w2s, h1T_s[:, cs], start=True, stop=True)
            nc.scalar.activation(out=eT_s[:, cs], in_=pm[:], func=AF.Identity,
                                 bias=b2sc, scale=1.0)

        for k in range(2):
            cs = bass.ts(k, 256)
            pm = PM.tile([64, 256], F32, tag="pm", name="pm_w1cn")
            nc.tensor.matmul(pm[:], w1cn, eT_n[:, cs], start=True, stop=True)
            nc.scalar.activation(out=Vrep[0:64, cs], in_=pm[:], func=AF.Identity,
                                 bias=b1c, scale=1.0)
            nc.vector.tensor_scalar_add(out=Vrep[64:128, cs], in0=pm[:], scalar1=b1c)
            pm = PM.tile([64, 256], F32, tag="pm", name="pm_w1cs")
            nc.tensor.matmul(pm[:], w1cs, eT_s[:, cs], start=True, stop=True)
            psplit = pm[:].rearrange("p (i g) -> p i g", g=2)
            nc.gpsimd.tensor_copy(out=U2[0:64, bass.ts(k, 128)], in_=psplit[:, :, 0])
            nc.gpsimd.tensor_copy(out=U2[64:128, bass.ts(k, 128)], in_=psplit[:, :, 1])

        # ---------- self_e (f32) / nb_e (bf16) via PE chunk transposes ------
        selfe, nbe = [], []
        for it in range(NT):
            pt = PT.tile([128, 128], F32R, tag="pt", name="pts", padded_shape=[128, 128])
            nc.tensor.transpose(pt[:, 0:64], eT_s[:, bass.ts(it, 128)], idf[0:64, 0:64])
            se = P.tile([128, H], F32, tag=f"selfe{it}")
            nc.gpsimd.tensor_copy(out=se[:], in_=pt[:, 0:64])
            selfe.append(se)
            ptn = PT.tile([128, 128], BF16, tag="pt", name="ptn", padded_shape=[128, 128])
            nc.tensor.transpose(ptn[:, 0:64], eT_n[:, bass.ts(it, 128)], idb[0:64, 0:64])
            ne = P.tile([128, H], BF16, tag=f"nbe{it}")
            nc.gpsimd.tensor_copy(out=ne[:], in_=ptn[:, 0:64])
            nbe.append(ne)

        # ---------- exp(0.2 sV) row -> [128, NT] per-partition scalars ------
        pm = PM.tile([64, 512], F32, tag="pm", name="pm_sv")
        nc.tensor.matmul(pm[:1, :], w2cb, Vrep[0:64, :], start=True, stop=True)
        sv_row = SM.tile([1, N], F32R, tag="sv_row")
        nc.scalar.activation(out=sv_row[:], in_=pm[:1, :], func=AF.Exp, scale=0.2)
        pesv = PT.tile([128, 128], F32R, tag="pt", name="pesv", padded_shape=[128, 128])
        for tq in range(NT):
            nc.tensor.transpose(pesv[:, tq:tq + 1], sv_row[:, bass.ts(tq, 128)],
                                idf[0:1, 0:1])
        esv = P.tile([128, NT], F32, tag="esv")
        nc.gpsimd.tensor_copy(out=esv[:], in_=pesv[:, 0:NT])

        # ---------- mask tiles: edges * notdiag * exp(0.2 sV_j) ----------
        masks = []
        for jt in range(NT):
            mj = P.tile([128, N], BF16, tag=f"mask{jt}", name=f"mask{jt}")
            nc.vector.tensor_scalar_mul(out=mj[:], in0=esbs[jt][:],
                                        scalar1=esv[:, jt:jt + 1])
            nc.vector.tensor_mul(out=mj[:, bass.ts(jt, 128)],
                                 in0=mj[:, bass.ts(jt, 128)], in1=ive[:])
            masks.append(mj)

        # ---------- main pass (software-pipelined) ----------
        ET = [P.tile([128, N], BF16, tag=f"ET{jt}", name=f"ET{jt}") for jt in range(NT)]
        pd = PD.tile([128, N], F32, tag="pd")
        pa_all = PA.tile([128, NT, H], F32, tag="pa_all")
        n_v = PAT.count("v")
        mm_total = 2 * n_v + (16 - n_v)
        ps_tiles = [None] * NT

        def emit_scores_group(it, c, ps):
            base = 64 * it + 32 * c
            mm_i = 0
            for sp in range(16):            # pass 1: bf16 slot-pairs
                if PAT[sp] != "v":
                    continue
                for tt in range(2):
                    s = 2 * sp + tt
                    rl = RL.tile([128, N], BF16, tag="relu")
                    nc.vector.tensor_scalar(out=rl[:], in0=Vrep[:],
                                            scalar1=U2[:, base + s:base + s + 1],
                                            scalar2=0.0, op0=OP.add, op1=OP.max)
                    nc.tensor.matmul(ps[bass.ts(c, 64), :], w2bd_sb[s], rl[:],
                                     start=(mm_i == 0), stop=(mm_i == mm_total - 1))
                    mm_i += 1
            for sp in range(16):            # pass 2: fp8 DoubleRow slot-pairs
                ch = PAT[sp]
                if ch == "v":
                    continue
                rl2 = R8.tile([128, 2 * N], FP8, tag="relu8")
                for tt, eng in enumerate(FP8_ENG[ch]):
                    p = base + 2 * sp + tt
                    seg = rl2[:, N * tt:N * (tt + 1)]
                    if eng == "A":
                        nc.scalar.activation(out=seg, in_=Vrep[:], func=AF.Relu,
                                             bias=U2[:, p:p + 1], scale=1.0)
                    elif eng == "P":
                        nc.gpsimd.tensor_scalar(out=seg, in0=Vrep[:],
                                                scalar1=U2[:, p:p + 1], scalar2=0.0,
                                                op0=OP.add, op1=OP.max)
                    else:
                        nc.vector.tensor_scalar(out=seg, in0=Vrep[:],
                                                scalar1=U2[:, p:p + 1], scalar2=0.0,
                                                op0=OP.add, op1=OP.max)
                nc.tensor.matmul(
                    ps[bass.ts(c, 64), :],
                    w2dr[:, bass.ts(sp, 128)].rearrange("p (t m) -> p t m", t=2),
                    rl2[:].rearrange("p (t j) -> p t j", t=2),
                    start=(mm_i == 0), stop=(mm_i == mm_total - 1),
                    perf_mode=DRMODE)
                mm_i += 1

        def emit_post(it):
            ps = ps_tiles[it]
            X = XE.tile([128, N], BF16, tag="X")
            nc.scalar.activation(out=X[:], in_=ps[:], func=AF.Exp)
            for jt in range(NT):
                px = PT.tile([128, 128], BF16, tag="pt", name="px2")
                nc.tensor.transpose(px[:], X[:, bass.ts(jt, 128)], idb[:])
                nc.vector.tensor_mul(out=ET[jt][:, bass.ts(it, 128)], in0=px[:],
                                     in1=masks[jt][:, bass.ts(it, 128)])
            for jt in range(NT):
                nc.tensor.matmul(pd[:1, bass.ts(it, 128)], onesc[:],
                                 ET[jt][:, bass.ts(it, 128)],
                                 start=(jt == 0), stop=(jt == NT - 1))
            for jt in range(NT):
                nc.tensor.matmul(pa_all[:, it, :], ET[jt][:, bass.ts(it, 128)],
                                 nbe[jt][:], start=(jt == 0), stop=(jt == NT - 1))
            # denom [1,128] row -> [128,1] per-partition scalars via K=1 PE
            # transpose (stays on-chip)
            den_row = SM.tile([1, 128], F32R, tag="den_row")
            nc.gpsimd.tensor_copy(out=den_row[:], in_=pd[:1, bass.ts(it, 128)])
            pden = PT.tile([128, 128], F32R, tag="pt", name="pden",
                           padded_shape=[128, 128])
            nc.tensor.transpose(pden[:, 0:1], den_row[:], idf[0:1, 0:1])
            gate = SM.tile([128, 1], F32, tag="gate", name="gate")
            nc.vector.tensor_single_scalar(out=gate[:], in_=pden[:, 0:1],
                                           scalar=1e-6, op=OP.is_gt)
            dsafe = SM.tile([128, 1], F32, tag="dsafe", name="dsafe")
            nc.vector.tensor_scalar_max(out=dsafe[:], in0=pden[:, 0:1], scalar1=1e-30)
            recipg = SM.tile([128, 1], F32, tag="recipg", name="recipg")
            nc.vector.reciprocal(out=recipg[:], in_=dsafe[:])
            sg = SM.tile([128, H], F32, tag="sg")
            nc.gpsimd.tensor_scalar_mul(out=sg[:], in0=selfe[it][:], scalar1=gate[:])
            nc.vector.tensor_mul(out=recipg[:], in0=recipg[:], in1=gate[:])
            ot = SM.tile([128, H], F32, tag="ot")
            nc.vector.scalar_tensor_tensor(out=ot[:], in0=pa_all[:, it, :],
                                           scalar=recipg[:], in1=sg[:],
                                           op0=OP.mult, op1=OP.add)
            nc.sync.dma_start(out=t["out"].ap()[bass.ts(it, 128), :], in_=ot[:])

        for it in range(NT):
            ps = PR.tile([128, N], F32, tag="psumR")
            ps_tiles[it] = ps
            emit_scores_group(it, 0, ps)
            if it >= 1:
                emit_post(it - 1)
            emit_scores_group(it, 1, ps)
        emit_post(NT - 1)


def _host_constants(inputs):
    f32 = np.float32
    bf = ml_dtypes.bfloat16
    f8 = ml_dtypes.float8_e4m3
    H_ = H
    w2 = np.asarray(inputs["comb_w2"], f32)[:, 0]      # [H]
    w2v = (0.8 * w2).astype(f8).astype(f32)

    wpack = np.zeros((128, 192), f32)
    wpack[:, 0:64] = np.asarray(inputs["self_w1"], f32)
    wpack[0:64, 64:128] = np.asarray(inputs["self_w2"], f32)
    wpack[0:64, 128:192] = np.asarray(inputs["comb_w1"], f32)[:H_]
    bvec = np.stack([
        np.asarray(inputs["self_b1"], f32),
        np.asarray(inputs["nb_b1"], f32),
        np.asarray(inputs["self_b2"], f32),
        np.asarray(inputs["nb_b2"], f32),
        np.asarray(inputs["comb_b1"], f32),
    ], axis=1)

    bfpack = np.zeros((128, 640), f32)
    bfpack[:, 0:64] = np.asarray(inputs["nb_w1"], f32)
    bfpack[0:64, 64:128] = np.asarray(inputs["nb_w2"], f32)
    bfpack[0:64, 128:192] = np.asarray(inputs["comb_w1"], f32)[H_:]
    bfpack[0:64, 192] = w2
    bfpack[:, 193:321] = np.eye(128, dtype=f32)
    bfpack[:, 321:449] = 1.0 - np.eye(128, dtype=f32)
    bfpack[0:64, 449 + 62] = 0.8 * w2
    bfpack[64:128, 449 + 63] = 0.8 * w2

    w2drall = np.zeros((128, 2048), f32)
    for sp in range(16):
        base = sp * 128
        w2drall[0:64, base + 4 * sp] = w2v
        w2drall[64:128, base + 4 * sp + 1] = w2v
        w2drall[0:64, base + 64 + 4 * sp + 2] = w2v
        w2drall[64:128, base + 64 + 4 * sp + 3] = w2v

    return {
        "wpack": wpack,
        "bvec": bvec,
        "id_f32": np.eye(128, dtype=f32),
        "bfpack": bfpack.astype(bf),
        "w2drall": w2drall.astype(f8),
    }


def _build_fast_path(nc):
    """Cache a single jitted shard_map executable so repeat kernel() calls
    skip jax re-tracing (same lowering run_bass_kernel_spmd uses under axon)."""
    import jax
    from jax.sharding import Mesh, PartitionSpec
    from jax.experimental.shard_map import shard_map

    bass2jax.install_neuronx_cc_hook()
    pname = nc.partition_id_tensor.name if nc.partition_id_tensor else None
    in_names, out_names, out_avals = [], [], []
    for alloc in nc.m.functions[0].allocations:
        if not isinstance(alloc, mybir.MemoryLocationSet):
            continue
        name = alloc.memorylocations[0].name
        if alloc.kind == "ExternalInput":
            if name != pname:
                in_names.append(name)
        elif alloc.kind == "ExternalOutput":
            out_names.append(name)
            out_avals.append(jax.core.ShapedArray(tuple(alloc.tensor_shape),
                                                  mybir.dt.np(alloc.dtype)))
    all_names = in_names + out_names + ([pname] if pname else [])

    def _body(*args):
        operands = list(args)
        if pname is not None:
            operands.append(bass2jax.partition_id_tensor())
        return tuple(bass2jax._bass_exec_p.bind(
            *operands, out_avals=tuple(out_avals), in_names=tuple(all_names),
            out_names=tuple(out_names), lowering_input_output_aliases=(),
            sim_require_finite=True, sim_require_nnan=True, nc=nc))

    devices = jax.devices()[:NCORES]
    mesh = Mesh(np.asarray(devices), ("core",))
    n_io = len(in_names) + len(out_names)
    sharded = jax.jit(
        shard_map(_body, mesh=mesh, in_specs=(PartitionSpec("core"),) * n_io,
                  out_specs=(PartitionSpec("core"),) * len(out_names),
                  check_rep=False),
        keep_unused=True,
    )
    return sharded, in_names, out_names, out_avals


def kernel(**inputs):
    first = "nc" not in _CACHE
    if first:
        _CACHE["nc"] = _build_module()
    nc = _CACHE["nc"]

    consts = _host_constants(inputs)
    nodes = np.asarray(inputs["nodes"], np.float32).reshape(B, N, D)
    edges = (np.asarray(inputs["edges"]) != 0).astype(np.uint8)

    in_maps = []
    for c in range(NCORES):
        m = dict(consts)
        m["nodes"] = np.ascontiguousarray(nodes[c])
        m["edges"] = edges[c]
        in_maps.append(m)

    if first:
        res = run_bass_kernel_spmd(nc, in_maps, core_ids=list(range(NCORES)))
        _CACHE["fast"] = _build_fast_path(nc)
        return np.stack([res.results[c]["out"] for c in range(NCORES)]).astype(np.float32)

    import jax
    sharded, in_names, out_names, out_avals = _CACHE["fast"]
    ckey = hash(tuple((k, v.tobytes()) for k, v in sorted(consts.items())))
    if _CACHE.get("ckey") != ckey:
        _CACHE["cdev"] = {
            n: jax.device_put(np.concatenate([np.asarray(in_maps[c][n])
                                              for c in range(NCORES)], axis=0))
            for n in in_names if n not in ("nodes", "edges")
        }
        _CACHE["zdev"] = [jax.device_put(np.zeros((NCORES * a.shape[0], *a.shape[1:]),
                                                  a.dtype)) for a in out_avals]
        _CACHE["ckey"] = ckey
    cdev = _CACHE["cdev"]
    concat_in = [cdev[n] if n in cdev else
                 np.concatenate([np.asarray(in_maps[c][n]) for c in range(NCORES)], axis=0)
                 for n in in_names]
    outs = sharded(*concat_in, *_CACHE["zdev"])
    i = out_names.index("out")
    return np.asarray(outs[i]).reshape(NCORES, N, H).astype(np.float32)


# revision 19
# speedup vs baseline: 1.3302x; 1.0639x over previous
"""GAT message-passing kernel for Trainium2 (8 NeuronCores, data-parallel over batch).

Math (per batch element b, derived from the reference nn.Module):
    x      = nodes.reshape(N, D)
    self_e = mlp2(x, self_*)                 # [N, H]
    nb_e   = mlp2(x, nb_*)                   # [N, H]
    U      = self_e @ comb_w1[:H]            # [N, H]  (i side)
    V      = nb_e @ comb_w1[H:] + comb_b1    # [N, H]  (j side)
    scores(i,j) = leaky(U_i + V_j) @ w2 + b2
                = 0.8*relu(U_i+V_j)@w2 + 0.2*(sU_i + sV_j) + const_i
    Softmax over j is invariant to per-i constants, so only
      s'(i,j) = 0.8*relu(U_i+V_j)@w2 + 0.2*sV_j  matters, and
      exp(s') factorizes as exp(0.8 relu(...)@w2) * exp(0.2 sV_j).
    E^T[j,i] = edges[j,i]*(j!=i)*exp(0.2 sV_j) * exp(0.8 relu(U_i+V_j)@w2)
    denom[i] = sum_j E^T[j,i]; gate = denom > eps; recip = gate/denom
    out[i]   = gate * (recip * (E^T)^T @ nb_e + self_e)
    (|scores| < 2, so exp needs no max-subtraction.)

Device mapping (one core per batch element). The pairwise stage uses the
transposed (g,h)-on-partitions layout: partitions = (i-parity g, h), free = j,
so one tensor_scalar(add,max)/activation(Relu,bias) op builds relu(V + U_i)
for TWO i's at once as a [128, 512] tile. Per 16 slot-pairs (one 64-row PSUM
column group), a pattern string assigns each slot-pair one of:
  'v'  two bf16 builds on DVE (4x perf mode, ~194ns) + two bf16 slot matmuls
       (512 rows * 1 cyc = ~213ns each) using shifted block-diagonal 0.8*w2
       windows;
  'a'/'p'/'V' two fp8e4m3 builds on ACT/Pool/DVE + ONE DoubleRow fp8 matmul
       covering both i-pairs in 256 cycles (~107ns) — 4x PE throughput per
       pair vs bf16;
  'h'/'w'/'x' mixed-engine fp8 builds (ACT+Pool / ACT+DVE / Pool+DVE) + DR.
fp8 relu tiles + fp8 0.8*w2 quantization costs ~8e-4 output rel err (checked
against the fp64 reference; budget is 2e-2).

MLP/U/V precompute runs in fp32r (1 cyc/row at >=256 free vs 4 for fp32) for
the self chain (self_e adds into the output, needs f32 accuracy) and bf16 for
the neighbor chain, both chunked by 256 columns so the first U2/Vrep columns
land early. Denominators, aggregation and output assembly are unchanged from
the bf16 scheme: exp straight out of PSUM, PE transposes, DVE mask-muls,
ones-matmul denoms, E^T @ nb_e aggregation, K=1 PE transpose for the
[1,128]->[128,1] denom scatter. The main loop is software-pipelined: the
post-stage (exp/ET/denom/agg/assembly) of i-tile it-1 is emitted between the
two column groups of i-tile it, which keeps the in-order DVE/ACT queues from
stalling on X(it-1).
"""

import os
import sys

sys.path.insert(0, "/opt/trn_rl_repo")

import numpy as np
import ml_dtypes

import concourse.bass as bass
import concourse.bacc as bacc
import concourse.tile as tile
from concourse import mybir, bass2jax
from concourse.bass_utils import run_bass_kernel_spmd

B, N, H, D = 8, 512, 64, 128
NCORES = 8
NT = N // 128          # 4 i/j tiles of 128
NPAIR = N // 2         # 256 i-pairs
F32 = mybir.dt.float32
F32R = mybir.dt.float32r
BF16 = mybir.dt.bfloat16
FP8 = mybir.dt.float8e4
U8 = mybir.dt.uint8

# Per-column-group slot-pair engine pattern (16 chars, order irrelevant):
# 'v' bf16 DVE; 'a' fp8 ACT; 'p' fp8 Pool; 'V' fp8 DVE; 'h' ACT+Pool;
# 'w' ACT+DVE; 'x' Pool+DVE.
PAIR_PATTERN = os.environ.get("GAT_PAIR_PATTERN", "vvvvvvvvvvaaahpp")

_CACHE = {}


def _build_module():
    nc = bacc.Bacc("TRN2", target_bir_lowering=False, debug=False, num_devices=NCORES)

    nodes = nc.dram_tensor("nodes", [N, D], F32R, kind="ExternalInput")
    edges = nc.dram_tensor("edges", [N, N], U8, kind="ExternalInput")
    wpack = nc.dram_tensor("wpack", [128, 320], F32R, kind="ExternalInput")
    bvec = nc.dram_tensor("bvec", [64, 5], F32, kind="ExternalInput")
    bfpack = nc.dram_tensor("bfpack", [128, 640], BF16, kind="ExternalInput")
    w2drall = nc.dram_tensor("w2drall", [128, 2048], FP8, kind="ExternalInput")

    out = nc.dram_tensor("out", [N, H], F32, kind="ExternalOutput")

    with tile.TileContext(nc) as tc:
        _emit(nc, tc, locals())
    nc.compile()
    return nc


def _emit(nc, tc, t):
    AF = mybir.ActivationFunctionType
    OP = mybir.AluOpType
    DRMODE = mybir.MatmulPerfMode.DoubleRow
    PAT = PAIR_PATTERN
    assert len(PAT) == 16 and all(c in "vapVhwx" for c in PAT), PAT
    # builds per fp8 slot-pair: (engine_t0, engine_t1)
    FP8_ENG = {"a": "AA", "p": "PP", "V": "DD", "h": "AP", "w": "AD", "x": "PD"}

    with (
        tc.tile_pool(name="persist", bufs=1) as P,
        tc.tile_pool(name="xin", bufs=2) as XW,
        tc.tile_pool(name="ework", bufs=3) as EW,
        tc.tile_pool(name="edges", bufs=4) as EB,
        tc.tile_pool(name="relu", bufs=12) as RL,
        tc.tile_pool(name="relu8", bufs=8) as R8,
        tc.tile_pool(name="xexp", bufs=2) as XE,
        tc.tile_pool(name="small", bufs=4) as SM,
        tc.tile_pool(name="psumR", bufs=2, space="PSUM") as PR,
        tc.tile_pool(name="psumT", bufs=2, space="PSUM") as PT,
        tc.tile_pool(name="psumM", bufs=2, space="PSUM") as PM,
        tc.tile_pool(name="psumD", bufs=1, space="PSUM") as PD,
        tc.tile_pool(name="psumA", bufs=1, space="PSUM") as PA,
    ):
        # ---------- input DMAs (merged; all on the idle SP queue) ----------
        xall = XW.tile([128, NT, D], F32R, name="xall", tag="xall")
        nc.sync.dma_start(out=xall[:],
                          in_=t["nodes"].ap().rearrange("(t p) d -> p t d", t=NT))
        xins = [xall[:, it, :] for it in range(NT)]
        wp = P.tile([128, 320], F32R, tag="wpack")
        nc.sync.dma_start(out=wp[:], in_=t["wpack"].ap())
        bv = P.tile([64, 5], F32, tag="bvec")
        nc.sync.dma_start(out=bv[:], in_=t["bvec"].ap())
        bp = P.tile([128, 640], BF16, tag="bfpack")
        nc.sync.dma_start(out=bp[:], in_=t["bfpack"].ap())
        w2dr = P.tile([128, 2048], FP8, tag="w2drall")
        nc.sync.dma_start(out=w2dr[:], in_=t["w2drall"].ap())
        esb_all = EB.tile([128, NT, N], U8, tag="edges_in", name="esb_all")
        nc.sync.dma_start(out=esb_all[:],
                          in_=t["edges"].ap().rearrange("(t p) j -> p t j", t=NT))
        esbs = [esb_all[:, jt, :] for jt in range(NT)]

        # ---------- constant views ----------
        w1s, w2s, w1cs = wp[:, 0:64], wp[0:64, 64:128], wp[0:64, 128:192]
        idf = wp[:, 192:320]
        b1s, b1n = bv[:, 0:1], bv[:, 1:2]
        b2sc, b2nc, b1c = bv[:, 2:3], bv[:, 3:4], bv[:, 4:5]
        w1n, w2n, w1cn = bp[:, 0:64], bp[0:64, 64:128], bp[0:64, 128:192]
        w2cb = bp[0:64, 192:193]
        idb = bp[:, 193:321]
        ive = bp[:, 321:449]
        w2bd_sb = [bp[:, 449 + 62 - 2 * s: 449 + 126 - 2 * s] for s in range(32)]
        onesc = P.tile([128, 1], BF16, tag="onesc")
        nc.gpsimd.memset(onesc[:], 1.0)

        # ---------- x -> x^T (f32r transposes: 1.5 cyc/row) ----------
        xT = P.tile([D, N], F32R, tag="xT")
        for it in range(NT):
            px = PT.tile([128, 128], F32R, tag="pt", name="px", padded_shape=[128, 128])
            nc.tensor.transpose(px[:], xins[it], idf)
            nc.gpsimd.tensor_copy(out=xT[:, bass.ts(it, 128)], in_=px[:])

        # ---------- tiny MLPs, chunked by 256 cols (h on partitions) --------
        # nb chain in bf16, self chain in f32r (self_e adds into the output).
        h1T_n = P.tile([H, N], BF16, tag="h1T_n")
        h1T_s = P.tile([H, N], F32R, tag="h1T_s")
        eT_n = P.tile([H, N], BF16, tag="eT_n")
        eT_s = P.tile([H, N], F32R, tag="eT_s")
        Vrep = P.tile([128, N], BF16, tag="Vrep")
        U2 = P.tile([128, NPAIR], F32, tag="U2")

        for k in range(2):
            cs = bass.ts(k, 256)
            pm = PM.tile([64, 256], F32, tag="pm", name="pm_w1n")
            nc.tensor.matmul(pm[:], w1n, xT[:, cs], start=True, stop=True)
            zn = EW.tile([H, 256], BF16, tag="zn", name="zn")
            nc.scalar.activation(out=zn[:], in_=pm[:], func=AF.Identity,
                                 bias=b1n, scale=1.0)
            nc.vector.scalar_tensor_tensor(out=h1T_n[:, cs], in0=zn[:], scalar=0.2,
                                           in1=zn[:], op0=OP.mult, op1=OP.max)
            pm = PM.tile([64, 256], F32, tag="pm", name="pm_w1s")
            nc.tensor.matmul(pm[:], w1s, xT[:, cs], start=True, stop=True)
            zs = EW.tile([H, 256], F32, tag="zs", name="zs")
            nc.scalar.activation(out=zs[:], in_=pm[:], func=AF.Identity,
                                 bias=b1s, scale=1.0)
            nc.vector.scalar_tensor_tensor(out=h1T_s[:, cs], in0=zs[:], scalar=0.2,
                                           in1=zs[:], op0=OP.mult, op1=OP.max)

        for k in range(2):
            cs = bass.ts(k, 256)
            pm = PM.tile([64, 256], F32, tag="pm", name="pm_w2n")
            nc.tensor.matmul(pm[:], w2n, h1T_n[:, cs], start=True, stop=True)
            nc.scalar.activation(out=eT_n[:, cs], in_=pm[:], func=AF.Identity,
                                 bias=b2nc, scale=1.0)
            pm = PM.tile([64, 256], F32, tag="pm", name="pm_w2s")
            nc.tensor.matmul(pm[:], w2s, h1T_s[:, cs], start=True, stop=True)
            nc.scalar.activation(out=eT_s[:, cs], in_=pm[:], func=AF.Identity,
                                 bias=b2sc, scale=1.0)

        for k in range(2):
            cs = bass.ts(k, 256)
            pm = PM.tile([64, 256], F32, tag="pm", name="pm_w1cn")
            nc.tensor.matmul(pm[:], w1cn, eT_n[:, cs], start=True, stop=True)
            nc.scalar.activation(out=Vrep[0:64, cs], in_=pm[:], func=AF.Identity,
                                 bias=b1c, scale=1.0)
            nc.gpsimd.tensor_scalar_add(out=Vrep[64:128, cs], in0=pm[:], scalar1=b1c)
            pm = PM.tile([64, 256], F32, tag="pm", name="pm_w1cs")
            nc.tensor.matmul(pm[:], w1cs, eT_s[:, cs], start=True, stop=True)
            psplit = pm[:].rearrange("p (i g) -> p i g", g=2)
            nc.gpsimd.tensor_copy(out=U2[0:64, bass.ts(k, 128)], in_=psplit[:, :, 0])
            nc.gpsimd.tensor_copy(out=U2[64:128, bass.ts(k, 128)], in_=psplit[:, :, 1])

        # ---------- self_e (f32) / nb_e (bf16) via PE chunk transposes ------
        selfe, nbe = [], []
        for it in range(NT):
            pt = PT.tile([128, 128], F32R, tag="pt", name="pts", padded_shape=[128, 128])
            nc.tensor.transpose(pt[:, 0:64], eT_s[:, bass.ts(it, 128)],
                                wp[0:64, 192:256])
            se = P.tile([128, H], F32, tag=f"selfe{it}")
            nc.gpsimd.tensor_copy(out=se[:], in_=pt[:, 0:64])
            selfe.append(se)
            ptn = PT.tile([128, 128], BF16, tag="pt", name="ptn", padded_shape=[128, 128])
            nc.tensor.transpose(ptn[:, 0:64], eT_n[:, bass.ts(it, 128)],
                                bp[0:64, 193:257])
            ne = P.tile([128, H], BF16, tag=f"nbe{it}")
            nc.gpsimd.tensor_copy(out=ne[:], in_=ptn[:, 0:64])
            nbe.append(ne)

        # ---------- exp(0.2 sV) row -> [128, NT] per-partition scalars ------
        pm = PM.tile([64, 512], F32, tag="pm", name="pm_sv")
        nc.tensor.matmul(pm[:1, :], w2cb, Vrep[0:64, :], start=True, stop=True)
        sv_row = SM.tile([1, N], F32R, tag="sv_row")
        nc.scalar.activation(out=sv_row[:], in_=pm[:1, :], func=AF.Exp, scale=0.2)
        pesv = PT.tile([128, 128], F32R, tag="pt", name="pesv", padded_shape=[128, 128])
        for tq in range(NT):
            nc.tensor.transpose(pesv[:, tq:tq + 1], sv_row[:, bass.ts(tq, 128)],
                                wp[0:1, 192:193])
        esv = P.tile([128, NT], F32, tag="esv")
        nc.gpsimd.tensor_copy(out=esv[:], in_=pesv[:, 0:NT])

        # ---------- mask tiles: edges * notdiag * exp(0.2 sV_j) ----------
        # (allocated here; built inside the main loop so DVE's build stream
        # is not delayed at the start)
        masks = [P.tile([128, N], BF16, tag=f"mask{jt}", name=f"mask{jt}")
                 for jt in range(NT)]

        def emit_mask(jt):
            mj = masks[jt]
            nc.vector.tensor_scalar_mul(out=mj[:], in0=esbs[jt],
                                        scalar1=esv[:, jt:jt + 1])
            nc.vector.tensor_mul(out=mj[:, bass.ts(jt, 128)],
                                 in0=mj[:, bass.ts(jt, 128)], in1=ive[:])

        # ---------- main pass (software-pipelined) ----------
        ET = [P.tile([128, N], BF16, tag=f"ET{jt}", name=f"ET{jt}") for jt in range(NT)]
        pd = PD.tile([128, N], F32, tag="pd")
        pa_all = PA.tile([128, NT, H], F32, tag="pa_all")
        n_v = PAT.count("v")
        mm_total = 2 * n_v + (16 - n_v)
        ps_tiles = [None] * NT

        def emit_scores_group(it, c, ps):
            base = 64 * it + 32 * c
            mm_i = 0
            for sp in range(16):            # pass 1: bf16 slot-pairs
                if PAT[sp] != "v":
                    continue
                for tt in range(2):
                    s = 2 * sp + tt
                    rl = RL.tile([128, N], BF16, tag="relu")
                    nc.vector.tensor_scalar(out=rl[:], in0=Vrep[:],
                                            scalar1=U2[:, base + s:base + s + 1],
                                            scalar2=0.0, op0=OP.add, op1=OP.max)
                    nc.tensor.matmul(ps[bass.ts(c, 64), :], w2bd_sb[s], rl[:],
                                     start=(mm_i == 0), stop=(mm_i == mm_total - 1))
                    mm_i += 1
            for sp in range(16):            # pass 2: fp8 DoubleRow slot-pairs
                ch = PAT[sp]
                if ch == "v":
                    continue
                rl2 = R8.tile([128, 2 * N], FP8, tag="relu8")
                for tt, eng in enumerate(FP8_ENG[ch]):
                    p = base + 2 * sp + tt
                    seg = rl2[:, N * tt:N * (tt + 1)]
                    if eng == "A":
                        nc.scalar.activation(out=seg, in_=Vrep[:], func=AF.Relu,
                                             bias=U2[:, p:p + 1], scale=1.0)
                    elif eng == "P":
                        nc.gpsimd.tensor_scalar(out=seg, in0=Vrep[:],
                                                scalar1=U2[:, p:p + 1], scalar2=0.0,
                                                op0=OP.add, op1=OP.max)
                    else:
                        nc.vector.tensor_scalar(out=seg, in0=Vrep[:],
                                                scalar1=U2[:, p:p + 1], scalar2=0.0,
                                                op0=OP.add, op1=OP.max)
                nc.tensor.matmul(
                    ps[bass.ts(c, 64), :],
                    w2dr[:, bass.ts(sp, 128)].rearrange("p (t m) -> p t m", t=2),
                    rl2[:].rearrange("p (t j) -> p t j", t=2),
                    start=(mm_i == 0), stop=(mm_i == mm_total - 1),
                    perf_mode=DRMODE)
                mm_i += 1

        def emit_post(it):
            ps = ps_tiles[it]
            X = XE.tile([128, N], BF16, tag="X")
            nc.scalar.activation(out=X[:], in_=ps[:], func=AF.Exp)
            for jt in range(NT):
                px = PT.tile([128, 128], BF16, tag="pt", name="px2")
                nc.tensor.transpose(px[:], X[:, bass.ts(jt, 128)], idb[:])
                nc.vector.tensor_mul(out=ET[jt][:, bass.ts(it, 128)], in0=px[:],
                                     in1=masks[jt][:, bass.ts(it, 128)])
            for jt in range(NT):
                nc.tensor.matmul(pd[:1, bass.ts(it, 128)], onesc[:],
                                 ET[jt][:, bass.ts(it, 128)],
                                 start=(jt == 0), stop=(jt == NT - 1))
            for jt in range(NT):
                nc.tensor.matmul(pa_all[:, it, :], ET[jt][:, bass.ts(it, 128)],
                                 nbe[jt][:], start=(jt == 0), stop=(jt == NT - 1))
            # denom [1,128] row -> [128,1] per-partition scalars via K=1 PE
            # transpose (stays on-chip)
            den_row = SM.tile([1, 128], F32R, tag="den_row")
            nc.gpsimd.tensor_copy(out=den_row[:], in_=pd[:1, bass.ts(it, 128)])
            pden = PT.tile([128, 128], F32R, tag="pt", name="pden",
                           padded_shape=[128, 128])
            nc.tensor.transpose(pden[:, 0:1], den_row[:], wp[0:1, 192:193])
            gate = SM.tile([128, 1], F32, tag="gate", name="gate")
            nc.vector.tensor_single_scalar(out=gate[:], in_=pden[:, 0:1],
                                           scalar=1e-6, op=OP.is_gt)
            dsafe = SM.tile([128, 1], F32, tag="dsafe", name="dsafe")
            nc.vector.tensor_scalar_max(out=dsafe[:], in0=pden[:, 0:1], scalar1=1e-30)
            recipg = SM.tile([128, 1], F32, tag="recipg", name="recipg")
            nc.vector.reciprocal(out=recipg[:], in_=dsafe[:])
            sg = SM.tile([128, H], F32, tag="sg")
            nc.gpsimd.tensor_scalar_mul(out=sg[:], in0=selfe[it][:], scalar1=gate[:])
            nc.vector.tensor_mul(out=recipg[:], in0=recipg[:], in1=gate[:])
            ot = SM.tile([128, H], F32, tag="ot")
            nc.vector.scalar_tensor_tensor(out=ot[:], in0=pa_all[:, it, :],
                                           scalar=recipg[:], in1=sg[:],
                                           op0=OP.mult, op1=OP.add)
            nc.sync.dma_start(out=t["out"].ap()[bass.ts(it, 128), :], in_=ot[:])

        for it in range(NT):
            ps = PR.tile([128, N], F32, tag="psumR")
            ps_tiles[it] = ps
            emit_scores_group(it, 0, ps)
            if it == 0:
                emit_mask(0)
                emit_mask(1)
            else:
                emit_post(it - 1)
            emit_scores_group(it, 1, ps)
            if it == 0:
                emit_mask(2)
                emit_mask(3)
        emit_post(NT - 1)


def _host_constants(inputs):
    f32 = np.float32
    bf = ml_dtypes.bfloat16
    f8 = ml_dtypes.float8_e4m3
    H_ = H
    w2 = np.asarray(inputs["comb_w2"], f32)[:, 0]      # [H]
    w2v = (0.8 * w2).astype(f8).astype(f32)

    wpack = np.zeros((128, 320), f32)
    wpack[:, 0:64] = np.asarray(inputs["self_w1"], f32)
    wpack[0:64, 64:128] = np.asarray(inputs["self_w2"], f32)
    wpack[0:64, 128:192] = np.asarray(inputs["comb_w1"], f32)[:H_]
    wpack[:, 192:320] = np.eye(128, dtype=f32)
    bvec = np.stack([
        np.asarray(inputs["self_b1"], f32),
        np.asarray(inputs["nb_b1"], f32),
        np.asarray(inputs["self_b2"], f32),
        np.asarray(inputs["nb_b2"], f32),
        np.asarray(inputs["comb_b1"], f32),
    ], axis=1)

    bfpack = np.zeros((128, 640), f32)
    bfpack[:, 0:64] = np.asarray(inputs["nb_w1"], f32)
    bfpack[0:64, 64:128] = np.asarray(inputs["nb_w2"], f32)
    bfpack[0:64, 128:192] = np.asarray(inputs["comb_w1"], f32)[H_:]
    bfpack[0:64, 192] = w2
    bfpack[:, 193:321] = np.eye(128, dtype=f32)
    bfpack[:, 321:449] = 1.0 - np.eye(128, dtype=f32)
    bfpack[0:64, 449 + 62] = 0.8 * w2
    bfpack[64:128, 449 + 63] = 0.8 * w2

    w2drall = np.zeros((128, 2048), f32)
    for sp in range(16):
        base = sp * 128
        w2drall[0:64, base + 4 * sp] = w2v
        w2drall[64:128, base + 4 * sp + 1] = w2v
        w2drall[0:64, base + 64 + 4 * sp + 2] = w2v
        w2drall[64:128, base + 64 + 4 * sp + 3] = w2v

    return {
        "wpack": wpack,
        "bvec": bvec,
        "bfpack": bfpack.astype(bf),
        "w2drall": w2drall.astype(f8),
    }


def _build_fast_path(nc):
    """Cache a single jitted shard_map executable so repeat kernel() calls
    skip jax re-tracing (same lowering run_bass_kernel_spmd uses under axon)."""
    import jax
    from jax.sharding import Mesh, PartitionSpec
    from jax.experimental.shard_map import shard_map

    bass2jax.install_neuronx_cc_hook()
    pname = nc.partition_id_tensor.name if nc.partition_id_tensor else None
    in_names, out_names, out_avals = [], [], []
    for alloc in nc.m.functions[0].allocations:
        if not isinstance(alloc, mybir.MemoryLocationSet):
            continue
        name = alloc.memorylocations[0].name
        if alloc.kind == "ExternalInput":
            if name != pname:
                in_names.append(name)
        elif alloc.kind == "ExternalOutput":
            out_names.append(name)
            out_avals.append(jax.core.ShapedArray(tuple(alloc.tensor_shape),
                                                  mybir.dt.np(alloc.dtype)))
    all_names = in_names + out_names + ([pname] if pname else [])

    def _body(*args):
        operands = list(args)
        if pname is not None:
            operands.append(bass2jax.partition_id_tensor())
        return tuple(bass2jax._bass_exec_p.bind(
            *operands, out_avals=tuple(out_avals), in_names=tuple(all_names),
            out_names=tuple(out_names), lowering_input_output_aliases=(),
            sim_require_finite=True, sim_require_nnan=True, nc=nc))

    devices = jax.devices()[:NCORES]
    mesh = Mesh(np.asarray(devices), ("core",))
    n_io = len(in_names) + len(out_names)
    sharded = jax.jit(
        shard_map(_body, mesh=mesh, in_specs=(PartitionSpec("core"),) * n_io,
                  out_specs=(PartitionSpec("core"),) * len(out_names),
                  check_rep=False),
        keep_unused=True,
    )
    return sharded, in_names, out_names, out_avals


def kernel(**inputs):
    first = "nc" not in _CACHE
    if first:
        _CACHE["nc"] = _build_module()
    nc = _CACHE["nc"]

    consts = _host_constants(inputs)
    nodes = np.asarray(inputs["nodes"], np.float32).reshape(B, N, D)
    edges = (np.asarray(inputs["edges"]) != 0).astype(np.uint8)

    in_maps = []
    for c in range(NCORES):
        m = dict(consts)
        m["nodes"] = np.ascontiguousarray(nodes[c])
        m["edges"] = edges[c]
        in_maps.append(m)

    if first:
        res = run_bass_kernel_spmd(nc, in_maps, core_ids=list(range(NCORES)))
        _CACHE["fast"] = _build_fast_path(nc)
        return np.stack([res.results[c]["out"] for c in range(NCORES)]).astype(np.float32)

    import jax
    sharded, in_names, out_names, out_avals = _CACHE["fast"]
    ckey = hash(tuple((k, v.tobytes()) for k, v in sorted(consts.items())))
    if _CACHE.get("ckey") != ckey:
        _CACHE["cdev"] = {
            n: jax.device_put(np.concatenate([np.asarray(in_maps[c][n])
                                              for c in range(NCORES)], axis=0))
            for n in in_names if n not in ("nodes", "edges")
        }
        _CACHE["zdev"] = [jax.device_put(np.zeros((NCORES * a.shape[0], *a.shape[1:]),
                                                  a.dtype)) for a in out_avals]
        _CACHE["ckey"] = ckey
    cdev = _CACHE["cdev"]
    concat_in = [cdev[n] if n in cdev else
                 np.concatenate([np.asarray(in_maps[c][n]) for c in range(NCORES)], axis=0)
                 for n in in_names]
    outs = sharded(*concat_in, *_CACHE["zdev"])
    i = out_names.index("out")
    return np.asarray(outs[i]).reshape(NCORES, N, H).astype(np.float32)


# revision 29
# speedup vs baseline: 1.5065x; 1.1326x over previous
"""GAT message-passing kernel for Trainium2 (8 NeuronCores, data-parallel over batch).

Math (per batch element b, derived from the reference nn.Module):
    x      = nodes.reshape(N, D)
    self_e = mlp2(x, self_*)                 # [N, H]
    nb_e   = mlp2(x, nb_*)                   # [N, H]
    U      = self_e @ comb_w1[:H]            # [N, H]  (i side)
    V      = nb_e @ comb_w1[H:] + comb_b1    # [N, H]  (j side)
    scores(i,j) = leaky(U_i + V_j) @ w2 + b2
                = 0.8*relu(U_i+V_j)@w2 + 0.2*(sU_i + sV_j) + const_i
    Softmax over j is invariant to per-i constants, so only
      s'(i,j) = 0.8*relu(U_i+V_j)@w2 + 0.2*sV_j  matters, and
      exp(s') factorizes as exp(0.8 relu(...)@w2) * exp(0.2 sV_j).
    E^T[j,i] = edges[j,i]*(j!=i)*exp(0.2 sV_j) * exp(0.8 relu(U_i+V_j)@w2)
    denom[i] = sum_j E^T[j,i]; gate = denom > eps; recip = gate/denom
    out[i]   = gate * (recip * (E^T)^T @ nb_e + self_e)
    (|scores| < 2, so exp needs no max-subtraction.)

Device mapping (one core per batch element). The pairwise stage uses the
transposed (g,h)-on-partitions layout: partitions = (i-parity g, h), free = j,
so one tensor_scalar(add,max)/activation(Relu,bias) op builds relu(V + U_i)
for TWO i's at once as a [128, 512] tile. Per 16 slot-pairs (one 64-row PSUM
column group), a pattern string assigns each slot-pair one of:
  'v'  two bf16 builds on DVE (4x perf mode, ~194ns) + two bf16 slot matmuls
       (512 rows * 1 cyc = ~213ns each) using shifted block-diagonal 0.8*w2
       windows;
  'a'/'p'/'V' two fp8e4m3 builds on ACT/Pool/DVE + ONE DoubleRow fp8 matmul
       covering both i-pairs in 256 cycles (~107ns) — 4x PE throughput per
       pair vs bf16;
  'h'/'w'/'x' mixed-engine fp8 builds (ACT+Pool / ACT+DVE / Pool+DVE) + DR.
fp8 relu tiles + fp8 0.8*w2 quantization costs ~8e-4 output rel err (checked
against the fp64 reference; budget is 2e-2).

MLP/U/V precompute runs in fp32r (1 cyc/row at >=256 free vs 4 for fp32) for
the self chain (self_e adds into the output, needs f32 accuracy) and bf16 for
the neighbor chain, both chunked by 256 columns so the first U2/Vrep columns
land early. Denominators, aggregation and output assembly are unchanged from
the bf16 scheme: exp straight out of PSUM, PE transposes, DVE mask-muls,
ones-matmul denoms, E^T @ nb_e aggregation, K=1 PE transpose for the
[1,128]->[128,1] denom scatter. The main loop is software-pipelined: the
post-stage (exp/ET/denom/agg/assembly) of i-tile it-1 is emitted between the
two column groups of i-tile it, which keeps the in-order DVE/ACT queues from
stalling on X(it-1).
"""

import os
import sys

sys.path.insert(0, "/opt/trn_rl_repo")

import numpy as np
import ml_dtypes

import concourse.bass as bass
import concourse.bacc as bacc
import concourse.tile as tile
from concourse import mybir, bass2jax
from concourse.bass_utils import run_bass_kernel_spmd

B, N, H, D = 8, 512, 64, 128
NCORES = 8
NT = N // 128          # 4 i/j tiles of 128
NPAIR = N // 2         # 256 i-pairs
F32 = mybir.dt.float32
F32R = mybir.dt.float32r
BF16 = mybir.dt.bfloat16
U8 = mybir.dt.uint8

# Per-pair build-engine cycle: 'v' DVE (4x bf16, ~194ns), 'a' ACT (~612ns),
# 'p' Pool (~806ns). Ratio tuned so the three engines finish together.
PAIR_PATTERN = os.environ.get("GAT_PAIR_PATTERN", "vvavvpvvavvpvavvpvva")

_CACHE = {}


def _build_module():
    nc = bacc.Bacc("TRN2", target_bir_lowering=False, debug=False, num_devices=NCORES)

    nodes = nc.dram_tensor("nodes", [N, D], F32R, kind="ExternalInput")
    edges = nc.dram_tensor("edges", [N, N], U8, kind="ExternalInput")
    wpack = nc.dram_tensor("wpack", [128, 320], F32R, kind="ExternalInput")
    bvec = nc.dram_tensor("bvec", [64, 5], F32, kind="ExternalInput")
    bfpack = nc.dram_tensor("bfpack", [128, 640], BF16, kind="ExternalInput")

    out = nc.dram_tensor("out", [N, H], F32, kind="ExternalOutput")

    with tile.TileContext(nc) as tc:
        _emit(nc, tc, locals())
    nc.compile()
    return nc


def _emit(nc, tc, t):
    AF = mybir.ActivationFunctionType
    OP = mybir.AluOpType
    PAT = PAIR_PATTERN
    assert all(c in "vap" for c in PAT), PAT

    with (
        tc.tile_pool(name="persist", bufs=1) as P,
        tc.tile_pool(name="xin", bufs=2) as XW,
        tc.tile_pool(name="ework", bufs=3) as EW,
        tc.tile_pool(name="edges", bufs=4) as EB,
        tc.tile_pool(name="relu", bufs=12) as RL,
        tc.tile_pool(name="xexp", bufs=3) as XE,
        tc.tile_pool(name="small", bufs=4) as SM,
        tc.tile_pool(name="psumS", bufs=1, space="PSUM") as SC,
        tc.tile_pool(name="psumT", bufs=1, space="PSUM") as PT,
        tc.tile_pool(name="psumM", bufs=2, space="PSUM") as PM,
        tc.tile_pool(name="psumA", bufs=1, space="PSUM") as PA,
    ):
        # ---------- input DMAs (merged; all on the idle SP queue) ----------
        xall = XW.tile([128, NT, D], F32R, name="xall", tag="xall")
        nc.sync.dma_start(out=xall[:],
                          in_=t["nodes"].ap().rearrange("(t p) d -> p t d", t=NT))
        xins = [xall[:, it, :] for it in range(NT)]
        wp = P.tile([128, 320], F32R, tag="wpack")
        nc.sync.dma_start(out=wp[:], in_=t["wpack"].ap())
        bv = P.tile([64, 5], F32, tag="bvec")
        nc.sync.dma_start(out=bv[:], in_=t["bvec"].ap())
        bp = P.tile([128, 640], BF16, tag="bfpack")
        nc.sync.dma_start(out=bp[:], in_=t["bfpack"].ap())
        esb_all = EB.tile([128, NT, N], U8, tag="edges_in", name="esb_all")
        nc.sync.dma_start(out=esb_all[:],
                          in_=t["edges"].ap().rearrange("(t p) j -> p t j", t=NT))
        esbs = [esb_all[:, jt, :] for jt in range(NT)]

        # ---------- constant views ----------
        w1s, w2s, w1cs = wp[:, 0:64], wp[0:64, 64:128], wp[0:64, 128:192]
        idf = wp[:, 192:320]
        b1s, b1n = bv[:, 0:1], bv[:, 1:2]
        b2sc, b2nc, b1c = bv[:, 2:3], bv[:, 3:4], bv[:, 4:5]
        w1n, w2n, w1cn = bp[:, 0:64], bp[0:64, 64:128], bp[0:64, 128:192]
        w2cb = bp[0:64, 192:193]
        idb = bp[:, 193:321]
        ive = bp[:, 321:449]
        w2pair = bp[:, 511:513]     # [128, 2]: col0 = 0.8*w2 on g0, col1 on g1

        # ---------- x -> x^T (f32r transposes: 1.5 cyc/row) ----------
        xT = P.tile([D, N], F32R, tag="xT")
        for it in range(NT):
            px = PT.tile([128, 128], F32R, tag="pt", name="px", padded_shape=[128, 128])
            nc.tensor.transpose(px[:], xins[it], idf)
            nc.gpsimd.tensor_copy(out=xT[:, bass.ts(it, 128)], in_=px[:])

        # ---------- tiny MLPs, chunked by 256 cols (h on partitions) --------
        # nb chain in bf16, self chain in f32r (self_e adds into the output).
        h1T_n = P.tile([H, N], BF16, tag="h1T_n")
        h1T_s = P.tile([H, N], F32R, tag="h1T_s")
        eT_n = P.tile([H, N], BF16, tag="eT_n")
        eT_s = P.tile([H, N], F32R, tag="eT_s")
        Vrep = P.tile([128, N], BF16, tag="Vrep")
        U2 = P.tile([128, NPAIR], F32, tag="U2")

        for k in range(2):
            cs = bass.ts(k, 256)
            pm = PM.tile([64, 256], F32, tag="pm", name="pm_w1n")
            nc.tensor.matmul(pm[:], w1n, xT[:, cs], start=True, stop=True)
            zn = EW.tile([H, 256], BF16, tag="zn", name="zn")
            nc.scalar.activation(out=zn[:], in_=pm[:], func=AF.Identity,
                                 bias=b1n, scale=1.0)
            nc.vector.scalar_tensor_tensor(out=h1T_n[:, cs], in0=zn[:], scalar=0.2,
                                           in1=zn[:], op0=OP.mult, op1=OP.max)
            pm = PM.tile([64, 256], F32, tag="pm", name="pm_w1s")
            nc.tensor.matmul(pm[:], w1s, xT[:, cs], start=True, stop=True)
            zs = EW.tile([H, 256], F32, tag="zs", name="zs")
            nc.scalar.activation(out=zs[:], in_=pm[:], func=AF.Identity,
                                 bias=b1s, scale=1.0)
            nc.vector.scalar_tensor_tensor(out=h1T_s[:, cs], in0=zs[:], scalar=0.2,
                                           in1=zs[:], op0=OP.mult, op1=OP.max)

        for k in range(2):
            cs = bass.ts(k, 256)
            pm = PM.tile([64, 256], F32, tag="pm", name="pm_w2n")
            nc.tensor.matmul(pm[:], w2n, h1T_n[:, cs], start=True, stop=True)
            nc.scalar.activation(out=eT_n[:, cs], in_=pm[:], func=AF.Identity,
                                 bias=b2nc, scale=1.0)
            pm = PM.tile([64, 256], F32, tag="pm", name="pm_w2s")
            nc.tensor.matmul(pm[:], w2s, h1T_s[:, cs], start=True, stop=True)
            nc.scalar.activation(out=eT_s[:, cs], in_=pm[:], func=AF.Identity,
                                 bias=b2sc, scale=1.0)

        for k in range(2):
            cs = bass.ts(k, 256)
            pm = PM.tile([64, 256], F32, tag="pm", name="pm_w1cn")
            nc.tensor.matmul(pm[:], w1cn, eT_n[:, cs], start=True, stop=True)
            nc.scalar.activation(out=Vrep[0:64, cs], in_=pm[:], func=AF.Identity,
                                 bias=b1c, scale=1.0)
            nc.gpsimd.tensor_scalar_add(out=Vrep[64:128, cs], in0=pm[:], scalar1=b1c)
            pm = PM.tile([64, 256], F32, tag="pm", name="pm_w1cs")
            nc.tensor.matmul(pm[:], w1cs, eT_s[:, cs], start=True, stop=True)
            psplit = pm[:].rearrange("p (i g) -> p i g", g=2)
            nc.gpsimd.tensor_copy(out=U2[0:64, bass.ts(k, 128)], in_=psplit[:, :, 0])
            nc.gpsimd.tensor_copy(out=U2[64:128, bass.ts(k, 128)], in_=psplit[:, :, 1])

        # ---------- self_e (f32) / nb_e+ones (bf16) via PE chunk transposes --
        selfe, nbe_aug = [], []
        for it in range(NT):
            pt = PT.tile([128, 128], F32R, tag="pt", name="pts", padded_shape=[128, 128])
            nc.tensor.transpose(pt[:, 0:64], eT_s[:, bass.ts(it, 128)],
                                wp[0:64, 192:256])
            se = P.tile([128, H], F32, tag=f"selfe{it}")
            nc.gpsimd.tensor_copy(out=se[:], in_=pt[:, 0:64])
            selfe.append(se)
            ptn = PT.tile([128, 128], BF16, tag="pt", name="ptn", padded_shape=[128, 128])
            nc.tensor.transpose(ptn[:, 0:64], eT_n[:, bass.ts(it, 128)],
                                bp[0:64, 193:257])
            # col 64 = 1.0: the agg matmul then also produces the softmax
            # denominator as output column 64 (no separate ones-matmuls).
            ne = P.tile([128, H + 1], BF16, tag=f"nbe{it}")
            nc.gpsimd.tensor_copy(out=ne[:, 0:64], in_=ptn[:, 0:64])
            nc.gpsimd.memset(ne[:, 64:65], 1.0)
            nbe_aug.append(ne)

        # ---------- 0.2*sV row -> [128, NT] per-partition (j) scalars -------
        # (applied as the exp bias, so exp(score + 0.2 sV_j) comes out of ACT
        # in one op and the mask tiles stay binary)
        pm = PM.tile([64, 512], F32, tag="pm", name="pm_sv")
        nc.tensor.matmul(pm[:1, :], w2cb, Vrep[0:64, :], start=True, stop=True)
        sv_row = SM.tile([1, N], F32R, tag="sv_row")
        nc.scalar.activation(out=sv_row[:], in_=pm[:1, :], func=AF.Identity, scale=0.2)
        pesv = PT.tile([128, 128], F32R, tag="pt", name="pesv", padded_shape=[128, 128])
        for tq in range(NT):
            nc.tensor.transpose(pesv[:, tq:tq + 1], sv_row[:, bass.ts(tq, 128)],
                                wp[0:1, 192:193])
        svT = P.tile([128, NT], F32, tag="svT")
        nc.gpsimd.tensor_copy(out=svT[:], in_=pesv[:, 0:NT])

        # ---------- binary mask tiles: edges[j,i] * (j != i) ----------
        # (allocated here; built inside the main loop so DVE's build stream
        # is not delayed at the start)
        masks = [P.tile([128, N], BF16, tag=f"mask{jt}", name=f"mask{jt}")
                 for jt in range(NT)]

        def emit_mask(jt):
            mj = masks[jt]
            nc.vector.tensor_copy(out=mj[:], in_=esbs[jt])
            nc.vector.tensor_mul(out=mj[:, bass.ts(jt, 128)],
                                 in0=mj[:, bass.ts(jt, 128)], in1=ive[:])

        # ---------- main pass: j-major scores^T ----------
        # Pair p (i = 2p, 2p+1): its relu tile is the STATIONARY operand of 4
        # tiny matmuls (one per j-tile bank), rhs = the two 0.8*w2 columns ->
        # scores^T[j, 2p:2p+2] lands directly in [j, i] layout (no ET
        # transposes, denominators fused into agg).
        ET = [P.tile([128, N], BF16, tag=f"ET{jt}", name=f"ET{jt}") for jt in range(NT)]
        SCb = [SC.tile([128, N], F32, tag=f"sc{jt}", name=f"sc{jt}")
               for jt in range(NT)]
        pa_all = PA.tile([128, NT, H + 1], F32, tag="pa_all")

        def emit_pair(p):
            eng = PAT[p % len(PAT)]
            rl = RL.tile([128, N], BF16, tag="relu")
            if eng == "v":
                nc.vector.tensor_scalar(out=rl[:], in0=Vrep[:],
                                        scalar1=U2[:, p:p + 1], scalar2=0.0,
                                        op0=OP.add, op1=OP.max)
            elif eng == "a":
                nc.scalar.activation(out=rl[:], in_=Vrep[:], func=AF.Relu,
                                     bias=U2[:, p:p + 1], scale=1.0)
            else:
                nc.gpsimd.tensor_scalar(out=rl[:], in0=Vrep[:],
                                        scalar1=U2[:, p:p + 1], scalar2=0.0,
                                        op0=OP.add, op1=OP.max)
            # disjoint 2-column slices: every matmul is its own psum group, so
            # the bank is never mid-group and exp can read finished columns
            for jt in range(NT):
                nc.tensor.matmul(SCb[jt][:, 2 * p:2 * p + 2],
                                 rl[:, bass.ts(jt, 128)], w2pair,
                                 start=True, stop=True)

        def emit_post(its):
            lo, hi = 128 * its[0], 128 * (its[-1] + 1)
            for jt in range(NT):
                Xc = XE.tile([128, hi - lo], BF16, tag="X", name="Xc")
                nc.scalar.activation(out=Xc[:], in_=SCb[jt][:, lo:hi], func=AF.Exp,
                                     bias=svT[:, jt:jt + 1], scale=1.0)
                nc.vector.tensor_mul(out=ET[jt][:, lo:hi], in0=Xc[:],
                                     in1=masks[jt][:, lo:hi])
            for it in its:
                for jt in range(NT):
                    nc.tensor.matmul(pa_all[:, it, :], ET[jt][:, bass.ts(it, 128)],
                                     nbe_aug[jt][:], start=(jt == 0),
                                     stop=(jt == NT - 1))
                den = pa_all[:, it, H:H + 1]
                gate = SM.tile([128, 1], F32, tag="gate", name="gate")
                nc.vector.tensor_single_scalar(out=gate[:], in_=den,
                                               scalar=1e-6, op=OP.is_gt)
                dsafe = SM.tile([128, 1], F32, tag="dsafe", name="dsafe")
                nc.vector.tensor_scalar_max(out=dsafe[:], in0=den, scalar1=1e-30)
                recipg = SM.tile([128, 1], F32, tag="recipg", name="recipg")
                nc.vector.reciprocal(out=recipg[:], in_=dsafe[:])
                sg = SM.tile([128, H], F32, tag="sg")
                nc.gpsimd.tensor_scalar_mul(out=sg[:], in0=selfe[it][:],
                                            scalar1=gate[:])
                nc.vector.tensor_mul(out=recipg[:], in0=recipg[:], in1=gate[:])
                ot = SM.tile([128, H], F32, tag="ot")
                nc.vector.scalar_tensor_tensor(out=ot[:], in0=pa_all[:, it, 0:H],
                                               scalar=recipg[:], in1=sg[:],
                                               op0=OP.mult, op1=OP.add)
                nc.sync.dma_start(out=t["out"].ap()[bass.ts(it, 128), :], in_=ot[:])

        for p in range(NPAIR):
            emit_pair(p)
            if p == 34:
                emit_mask(0)
                emit_mask(1)
            elif p == 66:
                emit_mask(2)
                emit_mask(3)
            elif p == 150:
                emit_post([0, 1])
            elif p == 214:
                emit_post([2])
        emit_post([3])


def _host_constants(inputs):
    f32 = np.float32
    bf = ml_dtypes.bfloat16
    H_ = H
    w2 = np.asarray(inputs["comb_w2"], f32)[:, 0]      # [H]

    wpack = np.zeros((128, 320), f32)
    wpack[:, 0:64] = np.asarray(inputs["self_w1"], f32)
    wpack[0:64, 64:128] = np.asarray(inputs["self_w2"], f32)
    wpack[0:64, 128:192] = np.asarray(inputs["comb_w1"], f32)[:H_]
    wpack[:, 192:320] = np.eye(128, dtype=f32)
    bvec = np.stack([
        np.asarray(inputs["self_b1"], f32),
        np.asarray(inputs["nb_b1"], f32),
        np.asarray(inputs["self_b2"], f32),
        np.asarray(inputs["nb_b2"], f32),
        np.asarray(inputs["comb_b1"], f32),
    ], axis=1)

    bfpack = np.zeros((128, 640), f32)
    bfpack[:, 0:64] = np.asarray(inputs["nb_w1"], f32)
    bfpack[0:64, 64:128] = np.asarray(inputs["nb_w2"], f32)
    bfpack[0:64, 128:192] = np.asarray(inputs["comb_w1"], f32)[H_:]
    bfpack[0:64, 192] = w2
    bfpack[:, 193:321] = np.eye(128, dtype=f32)
    bfpack[:, 321:449] = 1.0 - np.eye(128, dtype=f32)
    bfpack[0:64, 449 + 62] = 0.8 * w2
    bfpack[64:128, 449 + 63] = 0.8 * w2

    return {
        "wpack": wpack,
        "bvec": bvec,
        "bfpack": bfpack.astype(bf),
    }


def _build_fast_path(nc):
    """Cache a single jitted shard_map executable so repeat kernel() calls
    skip jax re-tracing (same lowering run_bass_kernel_spmd uses under axon)."""
    import jax
    from jax.sharding import Mesh, PartitionSpec
    from jax.experimental.shard_map import shard_map

    bass2jax.install_neuronx_cc_hook()
    pname = nc.partition_id_tensor.name if nc.partition_id_tensor else None
    in_names, out_names, out_avals = [], [], []
    for alloc in nc.m.functions[0].allocations:
        if not isinstance(alloc, mybir.MemoryLocationSet):
            continue
        name = alloc.memorylocations[0].name
        if alloc.kind == "ExternalInput":
            if name != pname:
                in_names.append(name)
        elif alloc.kind == "ExternalOutput":
            out_names.append(name)
            out_avals.append(jax.core.ShapedArray(tuple(alloc.tensor_shape),
                                                  mybir.dt.np(alloc.dtype)))
    all_names = in_names + out_names + ([pname] if pname else [])

    def _body(*args):
        operands = list(args)
        if pname is not None:
            operands.append(bass2jax.partition_id_tensor())
        return tuple(bass2jax._bass_exec_p.bind(
            *operands, out_avals=tuple(out_avals), in_names=tuple(all_names),
            out_names=tuple(out_names), lowering_input_output_aliases=(),
            sim_require_finite=True, sim_require_nnan=True, nc=nc))

    devices = jax.devices()[:NCORES]
    mesh = Mesh(np.asarray(devices), ("core",))
    n_io = len(in_names) + len(out_names)
    sharded = jax.jit(
        shard_map(_body, mesh=mesh, in_specs=(PartitionSpec("core"),) * n_io,
                  out_specs=(PartitionSpec("core"),) * len(out_names),
                  check_rep=False),
        keep_unused=True,
    )
    return sharded, in_names, out_names, out_avals


def kernel(**inputs):
    first = "nc" not in _CACHE
    if first:
        _CACHE["nc"] = _build_module()
    nc = _CACHE["nc"]

    consts = _host_constants(inputs)
    nodes = np.asarray(inputs["nodes"], np.float32).reshape(B, N, D)
    edges = (np.asarray(inputs["edges"]) != 0).astype(np.uint8)

    in_maps = []
    for c in range(NCORES):
        m = dict(consts)
        m["nodes"] = np.ascontiguousarray(nodes[c])
        m["edges"] = edges[c]
        in_maps.append(m)

    if first:
        res = run_bass_kernel_spmd(nc, in_maps, core_ids=list(range(NCORES)))
        _CACHE["fast"] = _build_fast_path(nc)
        return np.stack([res.results[c]["out"] for c in range(NCORES)]).astype(np.float32)

    import jax
    sharded, in_names, out_names, out_avals = _CACHE["fast"]
    ckey = hash(tuple((k, v.tobytes()) for k, v in sorted(consts.items())))
    if _CACHE.get("ckey") != ckey:
        _CACHE["cdev"] = {
            n: jax.device_put(np.concatenate([np.asarray(in_maps[c][n])
                                              for c in range(NCORES)], axis=0))
            for n in in_names if n not in ("nodes", "edges")
        }
        _CACHE["zdev"] = [jax.device_put(np.zeros((NCORES * a.shape[0], *a.shape[1:]),
                                                  a.dtype)) for a in out_avals]
        _CACHE["ckey"] = ckey
    cdev = _CACHE["cdev"]
    concat_in = [cdev[n] if n in cdev else
                 np.concatenate([np.asarray(in_maps[c][n]) for c in range(NCORES)], axis=0)
                 for n in in_names]
    outs = sharded(*concat_in, *_CACHE["zdev"])
    i = out_names.index("out")
    return np.asarray(outs[i]).reshape(NCORES, N, H).astype(np.float32)
